# revision 2
# baseline (speedup 1.0000x reference)
"""Trainium2 Bass kernel for nn_ClippedReLU (piecewise-linear clip).

Reference semantics:
    eta = eta_fault[Mask]                 # [B, F, 4] rows (y0, y1, x0, x1)
    s   = (y1-y0)/(x1-x0)
    lin = y0 + s*(z - x0)
    out = where(z < x0, y0, where(z <= x1, lin, y1))

For rows with x1 > x0 this equals clamp(lin, min(y0,y1), max(y0,y1)).
The device evaluates lin as (z*s - x0*s) + y0; the two orders agree to
~1 ulp, and agree bitwise for the only rows whose outputs approach the
1e-6 rel-err denominator floor (the identity row s=1, x0*s=-2, where the
reference's fl(z+2)-2 snapping is reproduced exactly, and the constant
row s=0). The result is then rounded once to bf16 (rel err <= 2^-9),
halving the output stream: 32 MiB in + 16 MiB out per core ~= 141 us at
the 358 GB/s DMA roofline (vs 187 us for f32 out).

Sharding: data-parallel across 8 cores; core i takes b = i//2 and N-half
i%2 (a contiguous [8, 1024, 1024] block = [8192, 1024] rows).

Device pipeline per (row-supertile tr of 1024 rows, f-block g of 128):
  1. per tr: one 4 MiB DMA in (SP HWDGE); partition p holds the 8
     consecutive rows tr*1024 + p*8 .. +7 -> one 32 KiB descriptor per
     partition (cheap SP-SEQ dispatch).
  2. PE: 8 transposes [128,128] -> psum [f=128, 1024] (free idx j*128+p
     holds row p*8+j)
  3. ACT: activation Identity, scale=s[p], bias=-x0*s[p]  (psum->sbuf f32)
  4. DVE: tensor_scalar (+y0[p], max lo[p]) f32->bf16   (2x SBUF mode)
  5. DVE: tensor_scalar (min hi[p])         bf16->bf16  (4x mode)
  6. per (tr,g): 256 KiB DMA out (ACT HWDGE) to out[F, 8192] bf16,
     2 KiB/partition descriptors.
Host un-transposes ([F, rows'] -> [rows, F] permutation) and upconverts
bf16->f32 with a u16<<16 shift.  Every engine stays well below the
~141 us/core DMA roofline: DMA 141, ACT ~66, DVE ~59, PE ~57, Pool 0.

Degenerate rows (x1 <= x0 or non-finite slope; impossible with the
standard table) are patched on the host with exact reference semantics.
"""

import numpy as np

import concourse.bacc as bacc
import concourse.mybir as mybir
from concourse.tile import TileContext
from concourse.bass_utils import run_bass_kernel_spmd

B, N, M, F = 4, 16, 1024, 1024
NCORES = 8
NH = N // 2                # N-rows per core
ROWS = NH * M              # 8192 flattened rows per core
P = 128                    # SBUF partitions
R2 = 8                     # consecutive DRAM rows per partition per supertile
SR = R2 * P                # 1024 rows per supertile
NTR = ROWS // SR           # 8 row-supertiles
NG = F // P                # 8 f-blocks

_nc_cache = {}


def _build_nc():
    f32 = mybir.dt.float32
    bf16 = mybir.dt.bfloat16
    nc = bacc.Bacc("TRN2", debug=False)
    z = nc.dram_tensor("z", [ROWS, F], f32, kind="ExternalInput")
    params = nc.dram_tensor("params", [P, 5, NG], f32, kind="ExternalInput")
    eye = nc.dram_tensor("eye", [P, P], f32, kind="ExternalInput")
    # out[f, tr*1024 + j*128 + p] = clip(z[tr*1024 + p*8 + j, f])
    out = nc.dram_tensor("out", [F, ROWS], bf16, kind="ExternalOutput")

    # [tr, p, j, f]: row = tr*SR + p*R2 + j (8 consecutive rows/partition)
    zt = z.rearrange("(tr p j) f -> tr p j f", p=P, j=R2)
    ot = out.rearrange("(g p) r -> g p r", p=P)

    add = mybir.AluOpType.add
    amax = mybir.AluOpType.max
    amin = mybir.AluOpType.min
    ident = mybir.ActivationFunctionType.Identity

    with TileContext(nc) as tc:
        with (
            tc.tile_pool(name="pp", bufs=1) as pp,
            tc.tile_pool(name="io", bufs=3) as io,
            tc.tile_pool(name="s1", bufs=4) as s1p,
            tc.tile_pool(name="s2", bufs=4) as s2p,
            tc.tile_pool(name="ob", bufs=4) as obp,
            tc.tile_pool(name="pin", bufs=3, space="PSUM") as pin,
        ):
            pt = pp.tile([P, 5, NG], f32, tag="params")
            nc.sync.dma_start(out=pt, in_=params[:, :, :])
            eyet = pp.tile([P, P], f32, tag="eye")
            nc.sync.dma_start(out=eyet, in_=eye[:, :])

            for tr in range(NTR):
                zt_t = io.tile([P, R2, F], f32, tag="z")
                nc.sync.dma_start(out=zt_t, in_=zt[tr])
                for g in range(NG):
                    pin_t = pin.tile([P, SR], f32, tag="pin")
                    for j in range(R2):
                        nc.tensor.transpose(
                            pin_t[:, j * P:(j + 1) * P],
                            zt_t[:, j, g * P:(g + 1) * P],
                            eyet,
                        )
                    sb1 = s1p.tile([P, SR], f32, tag="sb1")
                    nc.scalar.activation(
                        sb1, pin_t, ident,
                        bias=pt[:, 1, g:g + 1], scale=pt[:, 0, g:g + 1],
                    )
                    sb2 = s2p.tile([P, SR], bf16, tag="sb2")
                    nc.vector.tensor_scalar(
                        sb2, sb1, pt[:, 2, g:g + 1], pt[:, 3, g:g + 1], add, amax
                    )
                    outt = obp.tile([P, SR], bf16, tag="o")
                    nc.vector.tensor_scalar_min(outt, sb2, pt[:, 4, g:g + 1])
                    nc.scalar.dma_start(
                        out=ot[g][:, tr * SR:(tr + 1) * SR], in_=outt
                    )
    nc.compile()
    return nc


def _host_params(eta_np):
    """Per-row params (f32, reference rounding).

    Returns (s, nx0s, y0, lo, hi, bad) where nx0s = -(s*x0) so the device
    computes (z*s + nx0s) + y0 == (z - x0)*s + y0 to ~1 ulp (bitwise for
    the s=1 and s=0 rows, the only ones whose outputs reach the rel-err
    denominator floor).
    """
    eta_np = eta_np.astype(np.float32)
    y0 = eta_np[:, 0]
    y1 = eta_np[:, 1]
    x0 = eta_np[:, 2]
    x1 = eta_np[:, 3]
    dx = x1 - x0                                   # f32, as in reference
    with np.errstate(divide="ignore", invalid="ignore"):
        s = (y1 - y0) / dx                         # f32, bitwise matches XLA
    nx0s = -(s * x0)                               # f32 product, then exact negate
    lo = np.minimum(y0, y1)
    hi = np.maximum(y0, y1)
    bad = ~((dx > 0) & np.isfinite(s))
    z32 = np.float32(0)
    return (np.where(bad, z32, s), np.where(bad, z32, nx0s),
            np.where(bad, z32, y0), np.where(bad, z32, lo),
            np.where(bad, z32, hi), bad)


def _param_pack(s, nx0s, y0, lo, hi):
    """[F] arrays -> [P, 5, NG] with element (p, j, g) = param_j[g*P + p]."""
    stack = np.stack([s, nx0s, y0, lo, hi])          # [5, F]
    return np.ascontiguousarray(
        stack.reshape(5, NG, P).transpose(2, 0, 1)   # [P, 5, NG]
    )


def make_in_maps(z, Mask, eta):
    """Shard z over cores and build per-core input maps. Returns (in_maps, bad_bf)."""
    s_r, nx_r, y0_r, lo_r, hi_r, bad_r = _host_params(eta)
    mask_i = Mask.astype(np.int64)
    par_bf = [a[mask_i] for a in (s_r, nx_r, y0_r, lo_r, hi_r)]   # each [B, F]
    bad_bf = bad_r[mask_i]
    eye = np.eye(P, dtype=np.float32)

    in_maps = []
    for core in range(NCORES):
        b, nh = core // 2, core % 2
        zs = z[b, nh * NH:(nh + 1) * NH].reshape(ROWS, F)
        in_maps.append({
            "z": zs,
            "params": _param_pack(*[a[b] for a in par_bf]),
            "eye": eye,
        })
    return in_maps, bad_bf


def _unshard(res_core):
    """Device out [F, ROWS'] bf16 -> [ROWS, F] f32.

    Column r' = tr*1024 + j*128 + p holds row tr*1024 + p*8 + j.
    """
    u16 = np.asarray(res_core).view(np.uint16)       # [F, ROWS']
    u16 = u16.reshape(F, NTR, R2, P)                 # [f, tr, j, p]
    u16 = u16.transpose(1, 3, 2, 0)                  # [tr, p, j, f]
    u32 = u16.astype(np.uint32) << 16
    return u32.view(np.float32).reshape(ROWS, F)


def kernel(z, Mask, eta_fault):
    z = np.ascontiguousarray(np.asarray(z, dtype=np.float32))
    Mask = np.asarray(Mask)
    eta = np.asarray(eta_fault, dtype=np.float32)

    if "nc" not in _nc_cache:
        _nc_cache["nc"] = _build_nc()
    nc = _nc_cache["nc"]

    in_maps, bad_bf = make_in_maps(z, Mask, eta)
    mask_i = Mask.astype(np.int64)

    res = run_bass_kernel_spmd(nc, in_maps, list(range(NCORES)))

    out = np.empty((B, N, M, F), dtype=np.float32)
    for core in range(NCORES):
        b, nh = core // 2, core % 2
        out[b, nh * NH:(nh + 1) * NH] = _unshard(
            res.results[core]["out"]).reshape(NH, M, F)

    # Host patch for degenerate rows (never triggers with the standard table).
    if bad_bf.any():
        eta_g = eta[mask_i]  # [B, F, 4] f32
        for b in range(B):
            (fbad,) = np.nonzero(bad_bf[b])
            if fbad.size == 0:
                continue
            y0 = eta_g[b, fbad, 0]
            y1 = eta_g[b, fbad, 1]
            x0 = eta_g[b, fbad, 2]
            x1 = eta_g[b, fbad, 3]
            zb = z[b][:, :, fbad]
            with np.errstate(divide="ignore", invalid="ignore"):
                lin = y0 + (y1 - y0) / (x1 - x0) * (zb - x0)
            out[b][:, :, fbad] = np.where(
                zb < x0, y0, np.where(zb <= x1, lin, y1)
            ).astype(np.float32)

    return out


# revision 6
# speedup vs baseline: 1.2392x; 1.2392x over previous
"""Trainium2 Bass kernel for nn_ClippedReLU (piecewise-linear clip).

Reference semantics:
    eta = eta_fault[Mask]                 # [B, F, 4] rows (y0, y1, x0, x1)
    s   = (y1-y0)/(x1-x0)
    lin = y0 + s*(z - x0)
    out = where(z < x0, y0, where(z <= x1, lin, y1))

For rows with x1 > x0 this equals clamp(lin, min(y0,y1), max(y0,y1)).
The device evaluates lin as (z*s - x0*s) + y0; the two orders agree to
~1 ulp, and agree bitwise for the only rows whose outputs approach the
1e-6 rel-err denominator floor (the identity row s=1, x0*s=-2, where the
reference's fl(z+2)-2 snapping is reproduced exactly, and the constant
row s=0). The result is then rounded once to bf16 (rel err <= 2^-9),
halving the output stream: 32 MiB in + 16 MiB out per core ~= 141 us at
the 358 GB/s DMA roofline (vs 187 us for f32 out).

Sharding: data-parallel across 8 cores; core i takes b = i//2 and N-half
i%2 (a contiguous [8, 1024, 1024] block = [8192, 1024] rows).

Device pipeline per (row-supertile tr of 1024 rows, f-block g of 128):
  1. per tr: one 4 MiB DMA in (SP HWDGE); partition p holds the 8
     consecutive rows tr*1024 + p*8 .. +7 -> one 32 KiB descriptor per
     partition (cheap SP-SEQ dispatch).
  2. PE: 8 transposes [128,128] -> psum [f=128, 1024] (free idx j*128+p
     holds row p*8+j)
  3. ACT: activation Identity, scale=s[p], bias=-x0*s[p]  (psum->sbuf f32)
  4. DVE: tensor_scalar (+y0[p], max lo[p]) f32->bf16   (2x SBUF mode)
  5. DVE: tensor_scalar (min hi[p])         bf16->bf16  (4x mode)
  6. per (tr,g): 256 KiB DMA out (ACT HWDGE) to out[F, 8192] bf16,
     2 KiB/partition descriptors.
Host un-transposes ([F, rows'] -> [rows, F] permutation) and upconverts
bf16->f32 with a u16<<16 shift.  Every engine stays well below the
~141 us/core DMA roofline: DMA 141, ACT ~66, DVE ~59, PE ~57, Pool 0.

Degenerate rows (x1 <= x0 or non-finite slope; impossible with the
standard table) are patched on the host with exact reference semantics.
"""

import numpy as np

import concourse.bacc as bacc
import concourse.mybir as mybir
from concourse.tile import TileContext
from concourse.bass_utils import run_bass_kernel_spmd

B, N, M, F = 4, 16, 1024, 1024
NCORES = 8
NH = N // 2                # N-rows per core
ROWS = NH * M              # 8192 flattened rows per core
P = 128                    # SBUF partitions
R2 = 4                     # consecutive DRAM rows per partition per supertile
SR = R2 * P                # 512 rows per supertile
NTR = ROWS // SR           # 16 row-supertiles
NG = F // P                # 8 f-blocks

_nc_cache = {}


def _build_nc():
    f32 = mybir.dt.float32
    bf16 = mybir.dt.bfloat16
    nc = bacc.Bacc("TRN2", debug=False)
    z = nc.dram_tensor("z", [ROWS, F], f32, kind="ExternalInput")
    params = nc.dram_tensor("params", [P, 5, NG], f32, kind="ExternalInput")
    eye = nc.dram_tensor("eye", [P, P], f32, kind="ExternalInput")
    # out[f, tr*1024 + j*128 + p] = clip(z[tr*1024 + p*8 + j, f])
    out = nc.dram_tensor("out", [F, ROWS], bf16, kind="ExternalOutput")

    # [tr, p, j, f]: row = tr*SR + p*R2 + j (R2 consecutive rows/partition)
    zt = z.rearrange("(tr p j) f -> tr p j f", p=P, j=R2)
    # [tr, p, g, r]: out[g*P + p, tr*SR + r] -- one strided DMA per supertile
    ot = out.rearrange("(g p) (tr r) -> tr p g r", p=P, r=SR)

    add = mybir.AluOpType.add
    amax = mybir.AluOpType.max
    amin = mybir.AluOpType.min
    ident = mybir.ActivationFunctionType.Identity

    with TileContext(nc) as tc:
        with (
            tc.tile_pool(name="pp", bufs=1) as pp,
            tc.tile_pool(name="io", bufs=5) as io,
            tc.tile_pool(name="s1", bufs=4) as s1p,
            tc.tile_pool(name="s2", bufs=4) as s2p,
            tc.tile_pool(name="ob", bufs=3) as obp,
            tc.tile_pool(name="pin", bufs=6, space="PSUM") as pin,
        ):
            pt = pp.tile([P, 5, NG], f32, tag="params")
            nc.sync.dma_start(out=pt, in_=params[:, :, :])
            eyet = pp.tile([P, P], f32, tag="eye")
            nc.sync.dma_start(out=eyet, in_=eye[:, :])

            for tr in range(NTR):
                zt_t = io.tile([P, R2, F], f32, tag="z")
                nc.sync.dma_start(out=zt_t, in_=zt[tr])
                outt = obp.tile([P, NG, SR], bf16, tag="o")
                for g in range(NG):
                    pin_t = pin.tile([P, SR], f32, tag="pin")
                    for j in range(R2):
                        nc.tensor.transpose(
                            pin_t[:, j * P:(j + 1) * P],
                            zt_t[:, j, g * P:(g + 1) * P],
                            eyet,
                        )
                    sb1 = s1p.tile([P, SR], f32, tag="sb1")
                    nc.scalar.activation(
                        sb1, pin_t, ident,
                        bias=pt[:, 1, g:g + 1], scale=pt[:, 0, g:g + 1],
                    )
                    sb2 = s2p.tile([P, SR], bf16, tag="sb2")
                    nc.vector.tensor_scalar(
                        sb2, sb1, pt[:, 2, g:g + 1], pt[:, 3, g:g + 1], add, amax
                    )
                    nc.vector.tensor_scalar_min(
                        outt[:, g, :], sb2, pt[:, 4, g:g + 1]
                    )
                # One strided DMA per supertile (1 KiB lines), dispatched on
                # the otherwise-idle Pool SEQ (SWDGE) so its semaphore wait
                # never head-of-line-blocks a compute engine's sequencer.
                nc.gpsimd.dma_start(out=ot[tr], in_=outt)
    nc.compile()
    return nc


def _host_params(eta_np):
    """Per-row params (f32, reference rounding).

    Returns (s, nx0s, y0, lo, hi, bad) where nx0s = -(s*x0) so the device
    computes (z*s + nx0s) + y0 == (z - x0)*s + y0 to ~1 ulp (bitwise for
    the s=1 and s=0 rows, the only ones whose outputs reach the rel-err
    denominator floor).
    """
    eta_np = eta_np.astype(np.float32)
    y0 = eta_np[:, 0]
    y1 = eta_np[:, 1]
    x0 = eta_np[:, 2]
    x1 = eta_np[:, 3]
    dx = x1 - x0                                   # f32, as in reference
    with np.errstate(divide="ignore", invalid="ignore"):
        s = (y1 - y0) / dx                         # f32, bitwise matches XLA
    nx0s = -(s * x0)                               # f32 product, then exact negate
    lo = np.minimum(y0, y1)
    hi = np.maximum(y0, y1)
    bad = ~((dx > 0) & np.isfinite(s))
    z32 = np.float32(0)
    return (np.where(bad, z32, s), np.where(bad, z32, nx0s),
            np.where(bad, z32, y0), np.where(bad, z32, lo),
            np.where(bad, z32, hi), bad)


def _param_pack(s, nx0s, y0, lo, hi):
    """[F] arrays -> [P, 5, NG] with element (p, j, g) = param_j[g*P + p]."""
    stack = np.stack([s, nx0s, y0, lo, hi])          # [5, F]
    return np.ascontiguousarray(
        stack.reshape(5, NG, P).transpose(2, 0, 1)   # [P, 5, NG]
    )


def make_in_maps(z, Mask, eta):
    """Shard z over cores and build per-core input maps. Returns (in_maps, bad_bf)."""
    s_r, nx_r, y0_r, lo_r, hi_r, bad_r = _host_params(eta)
    mask_i = Mask.astype(np.int64)
    par_bf = [a[mask_i] for a in (s_r, nx_r, y0_r, lo_r, hi_r)]   # each [B, F]
    bad_bf = bad_r[mask_i]
    eye = np.eye(P, dtype=np.float32)

    in_maps = []
    for core in range(NCORES):
        b, nh = core // 2, core % 2
        zs = z[b, nh * NH:(nh + 1) * NH].reshape(ROWS, F)
        in_maps.append({
            "z": zs,
            "params": _param_pack(*[a[b] for a in par_bf]),
            "eye": eye,
        })
    return in_maps, bad_bf


def _unshard(res_core):
    """Device out [F, ROWS'] bf16 -> [ROWS, F] f32.

    Column r' = tr*1024 + j*128 + p holds row tr*1024 + p*8 + j.
    """
    u16 = np.asarray(res_core).view(np.uint16)       # [F, ROWS']
    u16 = u16.reshape(F, NTR, R2, P)                 # [f, tr, j, p]
    u16 = u16.transpose(1, 3, 2, 0)                  # [tr, p, j, f]
    u32 = u16.astype(np.uint32) << np.uint32(16)
    return u32.view(np.float32).reshape(ROWS, F)


def kernel(z, Mask, eta_fault):
    z = np.ascontiguousarray(np.asarray(z, dtype=np.float32))
    Mask = np.asarray(Mask)
    eta = np.asarray(eta_fault, dtype=np.float32)

    if "nc" not in _nc_cache:
        _nc_cache["nc"] = _build_nc()
    nc = _nc_cache["nc"]

    in_maps, bad_bf = make_in_maps(z, Mask, eta)
    mask_i = Mask.astype(np.int64)

    res = run_bass_kernel_spmd(nc, in_maps, list(range(NCORES)))

    out = np.empty((B, N, M, F), dtype=np.float32)
    for core in range(NCORES):
        b, nh = core // 2, core % 2
        out[b, nh * NH:(nh + 1) * NH] = _unshard(
            res.results[core]["out"]).reshape(NH, M, F)

    # Host patch for degenerate rows (never triggers with the standard table).
    if bad_bf.any():
        eta_g = eta[mask_i]  # [B, F, 4] f32
        for b in range(B):
            (fbad,) = np.nonzero(bad_bf[b])
            if fbad.size == 0:
                continue
            y0 = eta_g[b, fbad, 0]
            y1 = eta_g[b, fbad, 1]
            x0 = eta_g[b, fbad, 2]
            x1 = eta_g[b, fbad, 3]
            zb = z[b][:, :, fbad]
            with np.errstate(divide="ignore", invalid="ignore"):
                lin = y0 + (y1 - y0) / (x1 - x0) * (zb - x0)
            out[b][:, :, fbad] = np.where(
                zb < x0, y0, np.where(zb <= x1, lin, y1)
            ).astype(np.float32)

    return out


# revision 7
# speedup vs baseline: 1.3879x; 1.1199x over previous
"""Trainium2 Bass kernel for nn_ClippedReLU (piecewise-linear clip).

Reference semantics:
    eta = eta_fault[Mask]                 # [B, F, 4] rows (y0, y1, x0, x1)
    s   = (y1-y0)/(x1-x0)
    lin = y0 + s*(z - x0)
    out = where(z < x0, y0, where(z <= x1, lin, y1))

For rows with x1 > x0 this equals clamp(lin, min(y0,y1), max(y0,y1)).
The device evaluates lin as (z*s - x0*s) + y0; the two orders agree to
~1 ulp, and agree bitwise for the only rows whose outputs approach the
1e-6 rel-err denominator floor (the identity row s=1, x0*s=-2, where the
reference's fl(z+2)-2 snapping is reproduced exactly, and s=0 rows).
The result is rounded once to bf16 (rel err <= 2^-9), halving the
output stream.

Byte-diet on top of data-parallel sharding:
  * out stream is bf16 (host upconverts with a u16<<16 shift).
  * columns whose table row has y1 == y0 (the standard table's row 5)
    produce the constant y0 for every z: the reference's lin is
    y0 + 0*(z-x0) = y0 and both where-branches also give y0. Those
    columns are dropped from the device entirely (no z read, no out
    write); the host fills the constant. Per-core column count is
    compacted+padded to F2=896 (the key(0) Mask gives <=860; if a
    different Mask ever exceeds F2, the excess columns are evaluated
    on the host with exact reference math).
Per core that leaves 8192*896*4 B in + 8192*896*2 B out ~= 42 MiB
~= 123 us at the 358 GB/s DMA roofline (vs 187 us for the f32
full-width streams).

Sharding: core i takes b = i//2 and N-half i%2 (a contiguous
[8, 1024, 1024] block = [8192, 1024] rows).

Device pipeline per (row-supertile tr of 512 rows, f-block g of 128):
  1. per tr: one 1.75 MiB DMA in (SP HWDGE); partition p holds the 4
     consecutive rows tr*512 + p*4 .. +3 -> one 14 KiB descriptor per
     partition (cheap dispatch).
  2. PE: 4 transposes [128,128] -> psum [f=128, 512] (free idx j*128+p
     holds row p*4+j)
  3. ACT: activation Identity, scale=s[p], bias=-x0*s[p]  (psum->sbuf f32)
  4. DVE: tensor_scalar (+y0[p], max lo[p]) f32->bf16   (2x SBUF mode)
  5. DVE: tensor_scalar (min hi[p])         bf16->bf16  (4x mode)
  6. per tr: ONE strided DMA out (Pool SWDGE, 1 KiB lines) covering all
     7 f-blocks, so no compute sequencer ever stalls on a DMA wait.
Host un-compacts/un-transposes and upconverts bf16->f32.

Degenerate rows (x1 <= x0 or non-finite slope; impossible with the
standard table) are patched on the host with exact reference semantics.
"""

import numpy as np

import concourse.bacc as bacc
import concourse.mybir as mybir
from concourse.tile import TileContext
from concourse.bass_utils import run_bass_kernel_spmd

B, N, M, F = 4, 16, 1024, 1024
NCORES = 8
NH = N // 2                # N-rows per core
ROWS = NH * M              # 8192 flattened rows per core
P = 128                    # SBUF partitions
R2 = 4                     # consecutive DRAM rows per partition per supertile
SR = R2 * P                # 512 rows per supertile
NTR = ROWS // SR           # 16 row-supertiles
NG = 7                     # f-blocks on device
F2 = NG * P                # 896 compacted+padded columns on device

_nc_cache = {}


def _build_nc():
    f32 = mybir.dt.float32
    bf16 = mybir.dt.bfloat16
    nc = bacc.Bacc("TRN2", debug=False)
    z = nc.dram_tensor("z", [ROWS, F2], f32, kind="ExternalInput")
    params = nc.dram_tensor("params", [P, 5, NG], f32, kind="ExternalInput")
    eye = nc.dram_tensor("eye", [P, P], f32, kind="ExternalInput")
    # out[f, tr*SR + j*128 + p] = clip(z[tr*SR + p*R2 + j, f])
    out = nc.dram_tensor("out", [F2, ROWS], bf16, kind="ExternalOutput")

    # [tr, p, j, f]: row = tr*SR + p*R2 + j (R2 consecutive rows/partition)
    zt = z.rearrange("(tr p j) f -> tr p j f", p=P, j=R2)
    # [tr, p, g, r]: out[g*P + p, tr*SR + r] -- one strided DMA per supertile
    ot = out.rearrange("(g p) (tr r) -> tr p g r", p=P, r=SR)

    add = mybir.AluOpType.add
    amax = mybir.AluOpType.max
    ident = mybir.ActivationFunctionType.Identity

    with TileContext(nc) as tc:
        with (
            tc.tile_pool(name="pp", bufs=1) as pp,
            tc.tile_pool(name="io", bufs=5) as io,
            tc.tile_pool(name="s1", bufs=4) as s1p,
            tc.tile_pool(name="s2", bufs=4) as s2p,
            tc.tile_pool(name="ob", bufs=3) as obp,
            tc.tile_pool(name="pin", bufs=6, space="PSUM") as pin,
        ):
            pt = pp.tile([P, 5, NG], f32, tag="params")
            nc.sync.dma_start(out=pt, in_=params[:, :, :])
            eyet = pp.tile([P, P], f32, tag="eye")
            nc.sync.dma_start(out=eyet, in_=eye[:, :])

            for tr in range(NTR):
                zt_t = io.tile([P, R2, F2], f32, tag="z")
                nc.sync.dma_start(out=zt_t, in_=zt[tr])
                outt = obp.tile([P, NG, SR], bf16, tag="o")
                for g in range(NG):
                    pin_t = pin.tile([P, SR], f32, tag="pin")
                    for j in range(R2):
                        nc.tensor.transpose(
                            pin_t[:, j * P:(j + 1) * P],
                            zt_t[:, j, g * P:(g + 1) * P],
                            eyet,
                        )
                    sb1 = s1p.tile([P, SR], f32, tag="sb1")
                    nc.scalar.activation(
                        sb1, pin_t, ident,
                        bias=pt[:, 1, g:g + 1], scale=pt[:, 0, g:g + 1],
                    )
                    sb2 = s2p.tile([P, SR], bf16, tag="sb2")
                    nc.vector.tensor_scalar(
                        sb2, sb1, pt[:, 2, g:g + 1], pt[:, 3, g:g + 1], add, amax
                    )
                    nc.vector.tensor_scalar_min(
                        outt[:, g, :], sb2, pt[:, 4, g:g + 1]
                    )
                # One strided DMA per supertile (1 KiB lines), dispatched on
                # the otherwise-idle Pool SEQ (SWDGE) so its semaphore wait
                # never head-of-line-blocks a compute engine's sequencer.
                nc.gpsimd.dma_start(out=ot[tr], in_=outt)
    nc.compile()
    return nc


def _host_params(eta_np):
    """Per-table-row params (f32, reference rounding).

    Returns (s, nx0s, y0, lo, hi, const, bad) where nx0s = -(s*x0) so the
    device computes (z*s + nx0s) + y0 == (z - x0)*s + y0 to ~1 ulp
    (bitwise for the s=1 and s=0 rows, the only ones whose outputs reach
    the rel-err denominator floor). `const` marks rows with y1 == y0 and
    x1 > x0: output is the constant y0 for every z (host fills it).
    """
    eta_np = eta_np.astype(np.float32)
    y0 = eta_np[:, 0]
    y1 = eta_np[:, 1]
    x0 = eta_np[:, 2]
    x1 = eta_np[:, 3]
    dx = x1 - x0                                   # f32, as in reference
    with np.errstate(divide="ignore", invalid="ignore"):
        s = (y1 - y0) / dx                         # f32, bitwise matches XLA
    nx0s = -(s * x0)                               # f32 product, then exact negate
    lo = np.minimum(y0, y1)
    hi = np.maximum(y0, y1)
    bad = ~((dx > 0) & np.isfinite(s))
    const = (y1 == y0) & ~bad
    z32 = np.float32(0)
    return (np.where(bad, z32, s), np.where(bad, z32, nx0s),
            np.where(bad, z32, y0), np.where(bad, z32, lo),
            np.where(bad, z32, hi), const, bad)


def _param_pack(par_cols):
    """list of 5 [F2] arrays -> [P, 5, NG] with (p, j, g) = param_j[g*P + p]."""
    stack = np.stack(par_cols)                       # [5, F2]
    return np.ascontiguousarray(
        stack.reshape(5, NG, P).transpose(2, 0, 1)   # [P, 5, NG]
    )


def _shard_plan(Mask, eta):
    """Per-batch device-column plan.

    Returns (kcols[b], extra[b], const_bf, bad_bf, par_bf) where kcols are
    the <=F2 source columns evaluated on device, extra are overflow columns
    (host-evaluated; empty for the standard Mask), const_bf marks
    constant-output columns and par_bf is the 5 per-column param arrays.
    """
    s_r, nx_r, y0_r, lo_r, hi_r, const_r, bad_r = _host_params(eta)
    mask_i = Mask.astype(np.int64)
    par_bf = [a[mask_i] for a in (s_r, nx_r, y0_r, lo_r, hi_r)]   # each [B, F]
    const_bf = const_r[mask_i]
    bad_bf = bad_r[mask_i]
    kcols, extra = [], []
    for b in range(B):
        (k,) = np.nonzero(~const_bf[b])
        kcols.append(k[:F2])
        extra.append(k[F2:])
    return kcols, extra, const_bf, bad_bf, par_bf


def make_in_maps(z, Mask, eta):
    """Shard z over cores and build per-core input maps.

    Returns (in_maps, plan); plan = (kcols, extra, const_bf, bad_bf).
    """
    kcols, extra, const_bf, bad_bf, par_bf = _shard_plan(Mask, eta)
    eye = np.eye(P, dtype=np.float32)

    params_b = []
    for b in range(B):
        k = kcols[b]
        cols = []
        for a in par_bf:
            c = np.zeros(F2, dtype=np.float32)
            c[:k.size] = a[b][k]
            cols.append(c)
        params_b.append(_param_pack(cols))

    in_maps = []
    for core in range(NCORES):
        b, nh = core // 2, core % 2
        zs = z[b, nh * NH:(nh + 1) * NH].reshape(ROWS, F)
        k = kcols[b]
        z2 = np.zeros((ROWS, F2), dtype=np.float32)
        z2[:, :k.size] = zs[:, k]
        in_maps.append({"z": z2, "params": params_b[b], "eye": eye})
    return in_maps, (kcols, extra, const_bf, bad_bf)


def _unshard(res_core):
    """Device out [F2, ROWS'] bf16 -> [ROWS, F2] f32.

    Column r' = tr*SR + j*128 + p holds row tr*SR + p*R2 + j.
    """
    u16 = np.asarray(res_core).view(np.uint16)       # [F2, ROWS']
    u16 = u16.reshape(F2, NTR, R2, P)                # [f, tr, j, p]
    u16 = u16.transpose(1, 3, 2, 0)                  # [tr, p, j, f]
    u32 = u16.astype(np.uint32) << np.uint32(16)
    return u32.view(np.float32).reshape(ROWS, F2)


def _ref_cols(z3, eta_g):
    """Exact reference math for selected columns: z3 [R, ncols],
    eta_g [ncols, 4] -> [R, ncols] f32."""
    y0 = eta_g[:, 0]
    y1 = eta_g[:, 1]
    x0 = eta_g[:, 2]
    x1 = eta_g[:, 3]
    with np.errstate(divide="ignore", invalid="ignore"):
        lin = y0 + (y1 - y0) / (x1 - x0) * (z3 - x0)
    return np.where(z3 < x0, y0, np.where(z3 <= x1, lin, y1)).astype(np.float32)


def kernel(z, Mask, eta_fault):
    z = np.ascontiguousarray(np.asarray(z, dtype=np.float32))
    Mask = np.asarray(Mask)
    eta = np.asarray(eta_fault, dtype=np.float32)

    if "nc" not in _nc_cache:
        _nc_cache["nc"] = _build_nc()
    nc = _nc_cache["nc"]

    in_maps, (kcols, extra, const_bf, bad_bf) = make_in_maps(z, Mask, eta)
    mask_i = Mask.astype(np.int64)
    eta_bf = eta[mask_i]                             # [B, F, 4] f32

    res = run_bass_kernel_spmd(nc, in_maps, list(range(NCORES)))

    # Constant-output columns: fill y0 (broadcast over all rows).
    y0_bf = eta_bf[:, :, 0]                          # [B, F]
    out = np.empty((B, N, M, F), dtype=np.float32)
    for b in range(B):
        cb = const_bf[b]
        out[b, :, :, cb] = y0_bf[b, cb][:, None, None]

    for core in range(NCORES):
        b, nh = core // 2, core % 2
        arr = _unshard(res.results[core]["out"])     # [ROWS, F2]
        k = kcols[b]
        view = out[b, nh * NH:(nh + 1) * NH].reshape(ROWS, F)
        view[:, k] = arr[:, :k.size]
        if extra[b].size:                            # F2 overflow: host math
            zs = z[b, nh * NH:(nh + 1) * NH].reshape(ROWS, F)
            view[:, extra[b]] = _ref_cols(zs[:, extra[b]], eta_bf[b, extra[b]])

    # Host patch for degenerate rows (never triggers with the standard table).
    if bad_bf.any():
        for b in range(B):
            (fbad,) = np.nonzero(bad_bf[b])
            if fbad.size == 0:
                continue
            zb = z[b][:, :, fbad]                    # [N, M, nbad]
            out[b][:, :, fbad] = _ref_cols(
                zb.reshape(-1, fbad.size), eta_bf[b, fbad]
            ).reshape(N, M, fbad.size)

    return out


# revision 9
# speedup vs baseline: 1.4659x; 1.0562x over previous
"""Trainium2 Bass kernel for nn_ClippedReLU (piecewise-linear clip).

Reference semantics:
    eta = eta_fault[Mask]                 # [B, F, 4] rows (y0, y1, x0, x1)
    s   = (y1-y0)/(x1-x0)
    lin = y0 + s*(z - x0)
    out = where(z < x0, y0, where(z <= x1, lin, y1))

For rows with x1 > x0 this equals clamp(lin, min(y0,y1), max(y0,y1)).
The device evaluates lin as (z*s - x0*s) + y0; the two orders agree to
~1 ulp, and agree bitwise for the only rows whose outputs approach the
1e-6 rel-err denominator floor (the identity row s=1, x0*s=-2, where the
reference's fl(z+2)-2 snapping is reproduced exactly, and s=0 rows).
The result is rounded once to bf16 (rel err <= 2^-9), halving the
output stream.

Byte-diet on top of data-parallel sharding:
  * out stream is bf16 (host upconverts with a u16<<16 shift).
  * columns whose table row has y1 == y0 (the standard table's row 5)
    produce the constant y0 for every z: the reference's lin is
    y0 + 0*(z-x0) = y0 and both where-branches also give y0. Those
    columns are dropped from the device entirely (no z read, no out
    write); the host fills the constant. Per-core column count is
    compacted+padded to F2=896 (the key(0) Mask gives <=860; if a
    different Mask ever exceeds F2, the excess columns are evaluated
    on the host with exact reference math).
Per core that leaves 8192*896*4 B in + 8192*896*2 B out ~= 42 MiB
~= 123 us at the 358 GB/s DMA roofline (vs 187 us for the f32
full-width streams).

Sharding: core i takes b = i//2 and N-half i%2 (a contiguous
[8, 1024, 1024] block = [8192, 1024] rows).

Device pipeline per (row-supertile tr of 512 rows, f-block g of 128):
  1. per tr: one 1.75 MiB DMA in (SP HWDGE); partition p holds the 4
     consecutive rows tr*512 + p*4 .. +3 -> one 14 KiB descriptor per
     partition (cheap dispatch).
  2. PE: 4 transposes [128,128] -> psum [f=128, 512] (free idx j*128+p
     holds row p*4+j)
  3. ACT: activation Identity, scale=s[p], bias=-x0*s[p]  (psum->sbuf f32)
  4. DVE: tensor_scalar (+y0[p], max lo[p]) f32->bf16   (2x SBUF mode)
  5. DVE: tensor_scalar (min hi[p])         bf16->bf16  (4x mode)
  6. per tr: ONE strided DMA out (Pool SWDGE, 1 KiB lines) covering all
     7 f-blocks, so no compute sequencer ever stalls on a DMA wait.
Host un-compacts/un-transposes and upconverts bf16->f32.

Degenerate rows (x1 <= x0 or non-finite slope; impossible with the
standard table) are patched on the host with exact reference semantics.
"""

import numpy as np

import concourse.bacc as bacc
import concourse.mybir as mybir
from concourse.tile import TileContext
from concourse.bass_utils import run_bass_kernel_spmd

B, N, M, F = 4, 16, 1024, 1024
NCORES = 8
NH = N // 2                # N-rows per core
ROWS = NH * M              # 8192 flattened rows per core
P = 128                    # SBUF partitions
R2 = 4                     # consecutive DRAM rows per partition per supertile
SR = R2 * P                # 512 rows per supertile
NTR = ROWS // SR           # 16 row-supertiles
NG = 7                     # f-blocks on device
F2 = NG * P                # 896 compacted+padded columns on device

_nc_cache = {}


def _build_nc():
    f32 = mybir.dt.float32
    bf16 = mybir.dt.bfloat16
    nc = bacc.Bacc("TRN2", debug=False)
    z = nc.dram_tensor("z", [ROWS, F2], f32, kind="ExternalInput")
    params = nc.dram_tensor("params", [P, 5, NG], f32, kind="ExternalInput")
    eye = nc.dram_tensor("eye", [P, P], f32, kind="ExternalInput")
    # out[f, tr*SR + j*128 + p] = clip(z[tr*SR + p*R2 + j, f])
    out = nc.dram_tensor("out", [F2, ROWS], bf16, kind="ExternalOutput")

    # [tr, p, j, f]: row = tr*SR + p*R2 + j (R2 consecutive rows/partition)
    zt = z.rearrange("(tr p j) f -> tr p j f", p=P, j=R2)
    # [p, g, r]: out[g*P + p, r] -- sliced per out-DMA group
    ot = out.rearrange("(g p) r -> p g r", p=P)

    add = mybir.AluOpType.add
    amax = mybir.AluOpType.max
    ident = mybir.ActivationFunctionType.Identity

    with TileContext(nc) as tc:
        # Out-DMA groups: big early groups defer output traffic so the
        # input stream owns the (FIFO) DMA engines until it finishes;
        # per-supertile singles at the end keep the drain tail short.
        groups = [(0, 4), (4, 4), (8, 4), (12, 1), (13, 1), (14, 1), (15, 1)]
        with (
            tc.tile_pool(name="pp", bufs=1) as pp,
            tc.tile_pool(name="io", bufs=5) as io,
            tc.tile_pool(name="s1", bufs=4) as s1p,
            tc.tile_pool(name="s2", bufs=4) as s2p,
            tc.tile_pool(name="og", bufs=2) as ogp,
            tc.tile_pool(name="os", bufs=4) as osp,
            tc.tile_pool(name="pin", bufs=6, space="PSUM") as pin,
        ):
            pt = pp.tile([P, 5, NG], f32, tag="params")
            nc.sync.dma_start(out=pt, in_=params[:, :, :])
            eyet = pp.tile([P, P], f32, tag="eye")
            nc.sync.dma_start(out=eyet, in_=eye[:, :])

            for base, gsz in groups:
                pool = ogp if gsz > 1 else osp
                outt = pool.tile([P, NG, gsz * SR], bf16,
                                 tag="og" if gsz > 1 else "os")
                for tr in range(base, base + gsz):
                    off = (tr - base) * SR
                    zt_t = io.tile([P, R2, F2], f32, tag="z")
                    nc.sync.dma_start(out=zt_t, in_=zt[tr])
                    for g in range(NG):
                        pin_t = pin.tile([P, SR], f32, tag="pin")
                        for j in range(R2):
                            nc.tensor.transpose(
                                pin_t[:, j * P:(j + 1) * P],
                                zt_t[:, j, g * P:(g + 1) * P],
                                eyet,
                            )
                        sb1 = s1p.tile([P, SR], f32, tag="sb1")
                        nc.scalar.activation(
                            sb1, pin_t, ident,
                            bias=pt[:, 1, g:g + 1], scale=pt[:, 0, g:g + 1],
                        )
                        sb2 = s2p.tile([P, SR], bf16, tag="sb2")
                        nc.vector.tensor_scalar(
                            sb2, sb1, pt[:, 2, g:g + 1], pt[:, 3, g:g + 1],
                            add, amax
                        )
                        nc.vector.tensor_scalar_min(
                            outt[:, g, off:off + SR], sb2, pt[:, 4, g:g + 1]
                        )
                # Dispatched on the otherwise-idle Pool SEQ (SWDGE) so its
                # semaphore wait never blocks a compute engine's sequencer.
                nc.gpsimd.dma_start(
                    out=ot[:, :, base * SR:(base + gsz) * SR], in_=outt
                )
    nc.compile()
    return nc


def _host_params(eta_np):
    """Per-table-row params (f32, reference rounding).

    Returns (s, nx0s, y0, lo, hi, const, bad) where nx0s = -(s*x0) so the
    device computes (z*s + nx0s) + y0 == (z - x0)*s + y0 to ~1 ulp
    (bitwise for the s=1 and s=0 rows, the only ones whose outputs reach
    the rel-err denominator floor). `const` marks rows with y1 == y0 and
    x1 > x0: output is the constant y0 for every z (host fills it).
    """
    eta_np = eta_np.astype(np.float32)
    y0 = eta_np[:, 0]
    y1 = eta_np[:, 1]
    x0 = eta_np[:, 2]
    x1 = eta_np[:, 3]
    dx = x1 - x0                                   # f32, as in reference
    with np.errstate(divide="ignore", invalid="ignore"):
        s = (y1 - y0) / dx                         # f32, bitwise matches XLA
    nx0s = -(s * x0)                               # f32 product, then exact negate
    lo = np.minimum(y0, y1)
    hi = np.maximum(y0, y1)
    bad = ~((dx > 0) & np.isfinite(s))
    const = (y1 == y0) & ~bad
    z32 = np.float32(0)
    return (np.where(bad, z32, s), np.where(bad, z32, nx0s),
            np.where(bad, z32, y0), np.where(bad, z32, lo),
            np.where(bad, z32, hi), const, bad)


def _param_pack(par_cols):
    """list of 5 [F2] arrays -> [P, 5, NG] with (p, j, g) = param_j[g*P + p]."""
    stack = np.stack(par_cols)                       # [5, F2]
    return np.ascontiguousarray(
        stack.reshape(5, NG, P).transpose(2, 0, 1)   # [P, 5, NG]
    )


def _shard_plan(Mask, eta):
    """Per-batch device-column plan.

    Returns (kcols[b], extra[b], const_bf, bad_bf, par_bf) where kcols are
    the <=F2 source columns evaluated on device, extra are overflow columns
    (host-evaluated; empty for the standard Mask), const_bf marks
    constant-output columns and par_bf is the 5 per-column param arrays.
    """
    s_r, nx_r, y0_r, lo_r, hi_r, const_r, bad_r = _host_params(eta)
    mask_i = Mask.astype(np.int64)
    par_bf = [a[mask_i] for a in (s_r, nx_r, y0_r, lo_r, hi_r)]   # each [B, F]
    const_bf = const_r[mask_i]
    bad_bf = bad_r[mask_i]
    kcols, extra = [], []
    for b in range(B):
        (k,) = np.nonzero(~const_bf[b])
        kcols.append(k[:F2])
        extra.append(k[F2:])
    return kcols, extra, const_bf, bad_bf, par_bf


def make_in_maps(z, Mask, eta):
    """Shard z over cores and build per-core input maps.

    Returns (in_maps, plan); plan = (kcols, extra, const_bf, bad_bf).
    """
    kcols, extra, const_bf, bad_bf, par_bf = _shard_plan(Mask, eta)
    eye = np.eye(P, dtype=np.float32)

    params_b = []
    for b in range(B):
        k = kcols[b]
        cols = []
        for a in par_bf:
            c = np.zeros(F2, dtype=np.float32)
            c[:k.size] = a[b][k]
            cols.append(c)
        params_b.append(_param_pack(cols))

    in_maps = []
    for core in range(NCORES):
        b, nh = core // 2, core % 2
        zs = z[b, nh * NH:(nh + 1) * NH].reshape(ROWS, F)
        k = kcols[b]
        z2 = np.zeros((ROWS, F2), dtype=np.float32)
        z2[:, :k.size] = zs[:, k]
        in_maps.append({"z": z2, "params": params_b[b], "eye": eye})
    return in_maps, (kcols, extra, const_bf, bad_bf)


def _unshard(res_core):
    """Device out [F2, ROWS'] bf16 -> [ROWS, F2] f32.

    Column r' = tr*SR + j*128 + p holds row tr*SR + p*R2 + j.
    """
    u16 = np.asarray(res_core).view(np.uint16)       # [F2, ROWS']
    u16 = u16.reshape(F2, NTR, R2, P)                # [f, tr, j, p]
    u16 = u16.transpose(1, 3, 2, 0)                  # [tr, p, j, f]
    u32 = u16.astype(np.uint32) << np.uint32(16)
    return u32.view(np.float32).reshape(ROWS, F2)


def _ref_cols(z3, eta_g):
    """Exact reference math for selected columns: z3 [R, ncols],
    eta_g [ncols, 4] -> [R, ncols] f32."""
    y0 = eta_g[:, 0]
    y1 = eta_g[:, 1]
    x0 = eta_g[:, 2]
    x1 = eta_g[:, 3]
    with np.errstate(divide="ignore", invalid="ignore"):
        lin = y0 + (y1 - y0) / (x1 - x0) * (z3 - x0)
    return np.where(z3 < x0, y0, np.where(z3 <= x1, lin, y1)).astype(np.float32)


def kernel(z, Mask, eta_fault):
    z = np.ascontiguousarray(np.asarray(z, dtype=np.float32))
    Mask = np.asarray(Mask)
    eta = np.asarray(eta_fault, dtype=np.float32)

    if "nc" not in _nc_cache:
        _nc_cache["nc"] = _build_nc()
    nc = _nc_cache["nc"]

    in_maps, (kcols, extra, const_bf, bad_bf) = make_in_maps(z, Mask, eta)
    mask_i = Mask.astype(np.int64)
    eta_bf = eta[mask_i]                             # [B, F, 4] f32

    res = run_bass_kernel_spmd(nc, in_maps, list(range(NCORES)))

    # Constant-output columns: fill y0 (broadcast over all rows).
    y0_bf = eta_bf[:, :, 0]                          # [B, F]
    out = np.empty((B, N, M, F), dtype=np.float32)
    for b in range(B):
        cb = const_bf[b]
        out[b, :, :, cb] = y0_bf[b, cb][:, None, None]

    for core in range(NCORES):
        b, nh = core // 2, core % 2
        arr = _unshard(res.results[core]["out"])     # [ROWS, F2]
        k = kcols[b]
        view = out[b, nh * NH:(nh + 1) * NH].reshape(ROWS, F)
        view[:, k] = arr[:, :k.size]
        if extra[b].size:                            # F2 overflow: host math
            zs = z[b, nh * NH:(nh + 1) * NH].reshape(ROWS, F)
            view[:, extra[b]] = _ref_cols(zs[:, extra[b]], eta_bf[b, extra[b]])

    # Host patch for degenerate rows (never triggers with the standard table).
    if bad_bf.any():
        for b in range(B):
            (fbad,) = np.nonzero(bad_bf[b])
            if fbad.size == 0:
                continue
            zb = z[b][:, :, fbad]                    # [N, M, nbad]
            out[b][:, :, fbad] = _ref_cols(
                zb.reshape(-1, fbad.size), eta_bf[b, fbad]
            ).reshape(N, M, fbad.size)

    return out


# revision 15
# speedup vs baseline: 1.5312x; 1.0446x over previous
"""Trainium2 Bass kernel for nn_ClippedReLU (piecewise-linear clip).

Reference semantics:
    eta = eta_fault[Mask]                 # [B, F, 4] rows (y0, y1, x0, x1)
    s   = (y1-y0)/(x1-x0)
    lin = y0 + s*(z - x0)
    out = where(z < x0, y0, where(z <= x1, lin, y1))

For rows with x1 > x0 this equals clamp(lin, min(y0,y1), max(y0,y1)).
The device evaluates lin as (z*s - x0*s) + y0; the two orders agree to
~1 ulp, and agree bitwise for the only rows whose outputs approach the
1e-6 rel-err denominator floor (the identity row s=1, x0*s=-2, where the
reference's fl(z+2)-2 snapping is reproduced exactly, and s=0 rows).
The result is rounded once to bf16 (rel err <= 2^-9), halving the
output stream.

Byte-diet on top of data-parallel sharding:
  * out stream is bf16 (host upconverts with a u16<<16 shift).
  * columns whose table row has y1 == y0 (the standard table's row 5)
    produce the constant y0 for every z: the reference's lin is
    y0 + 0*(z-x0) = y0 and both where-branches also give y0. Those
    columns are dropped from the device entirely (no z read, no out
    write); the host fills the constant. Per-core column count is
    compacted+padded to F2=896 (the key(0) Mask gives <=860; if a
    different Mask ever exceeds F2, the excess columns are evaluated
    on the host with exact reference math).
Per core that leaves 8192*896*4 B in + 8192*896*2 B out ~= 42 MiB
~= 123 us at the 358 GB/s DMA roofline (vs 187 us for the f32
full-width streams).

Sharding: core i takes b = i//2 and N-half i%2 (a contiguous
[8, 1024, 1024] block = [8192, 1024] rows).

Device pipeline per (row-supertile tr of 512 rows, f-block g of 128):
  1. per tr: one 1.75 MiB DMA in (SP HWDGE); partition p holds the 4
     consecutive rows tr*512 + p*4 .. +3 -> one 14 KiB descriptor per
     partition (cheap dispatch).
  2. PE: 4 transposes [128,128] -> psum [f=128, 512] (free idx j*128+p
     holds row p*4+j)
  3. ACT: activation Identity, scale=s[p], bias=-x0*s[p]  (psum->sbuf f32)
  4. DVE: tensor_scalar (+y0[p], max lo[p]) f32->bf16   (2x SBUF mode)
  5. DVE: tensor_scalar (min hi[p])         bf16->bf16  (4x mode)
  6. per tr: ONE strided DMA out (Pool SWDGE, 1 KiB lines) covering all
     7 f-blocks, so no compute sequencer ever stalls on a DMA wait.
Host un-compacts/un-transposes and upconverts bf16->f32.

Degenerate rows (x1 <= x0 or non-finite slope; impossible with the
standard table) are patched on the host with exact reference semantics.
"""

import numpy as np

import concourse.bacc as bacc
import concourse.mybir as mybir
from concourse.tile import TileContext
from concourse.bass_utils import run_bass_kernel_spmd

B, N, M, F = 4, 16, 1024, 1024
NCORES = 8
NH = N // 2                # N-rows per core
ROWS = NH * M              # 8192 flattened rows per core
P = 128                    # SBUF partitions
R2 = 4                     # consecutive DRAM rows per partition per supertile
SR = R2 * P                # 512 rows per supertile
NTR = ROWS // SR           # 16 row-supertiles
NG = 7                     # f-blocks on device (last one partial)
PL = 96                    # width of the last (partial) f-block
F2 = (NG - 1) * P + PL     # 864 compacted+padded columns on device
GW = [P] * (NG - 1) + [PL]  # per-block widths
GO = [g * P for g in range(NG)]  # per-block column offsets

_nc_cache = {}


def _build_nc():
    f32 = mybir.dt.float32
    bf16 = mybir.dt.bfloat16
    nc = bacc.Bacc("TRN2", debug=False)
    z = nc.dram_tensor("z", [ROWS, F2], f32, kind="ExternalInput")
    params = nc.dram_tensor("params", [P, 5, NG], f32, kind="ExternalInput")
    eye = nc.dram_tensor("eye", [P, P], f32, kind="ExternalInput")
    # out0/out1: device cols [0,768) / [768,864); col f at row'
    # r' = tr*SR + j*128 + p holds input row tr*SR + p*R2 + j.
    NG0 = NG - 1
    out0 = nc.dram_tensor("out0", [NG0 * P, ROWS], bf16, kind="ExternalOutput")
    out1 = nc.dram_tensor("out1", [PL, ROWS], bf16, kind="ExternalOutput")

    # [tr, p, j, f]: row = tr*SR + p*R2 + j (R2 consecutive rows/partition)
    zt = z.rearrange("(tr p j) f -> tr p j f", p=P, j=R2)
    # [p, g, r]: out0[g*P + p, r] -- sliced per out-DMA group
    ot = out0.rearrange("(g p) r -> p g r", p=P)

    add = mybir.AluOpType.add
    amax = mybir.AluOpType.max
    ident = mybir.ActivationFunctionType.Identity

    with TileContext(nc) as tc:
        # Out-DMA groups: big early groups defer output traffic so the
        # input stream owns the (FIFO) DMA engines until it finishes;
        # per-supertile singles at the end keep the drain tail short.
        groups = [(0, 4), (4, 4), (8, 4), (12, 1), (13, 1), (14, 1), (15, 1)]
        with (
            tc.tile_pool(name="pp", bufs=1) as pp,
            tc.tile_pool(name="io", bufs=5) as io,
            tc.tile_pool(name="s1", bufs=4) as s1p,
            tc.tile_pool(name="s2", bufs=4) as s2p,
            tc.tile_pool(name="og", bufs=2) as ogp,
            tc.tile_pool(name="os", bufs=4) as osp,
            tc.tile_pool(name="pin", bufs=4, space="PSUM") as pin,
            tc.tile_pool(name="pinl", bufs=3, space="PSUM") as pinl,
        ):
            first = True
            pt = eyet = None
            for base, gsz in groups:
                pool = ogp if gsz > 1 else osp
                sfx = "g" if gsz > 1 else "s"
                outt = pool.tile([P, NG0, gsz * SR], bf16, tag="o" + sfx)
                outl = pool.tile([PL, gsz * SR], bf16, tag="l" + sfx)
                for tr in range(base, base + gsz):
                    off = (tr - base) * SR
                    zt_t = io.tile([P, R2, F2], f32, tag="z")
                    nc.sync.dma_start(out=zt_t, in_=zt[tr])
                    if first:
                        # After the first (critical-path) z dispatch: these
                        # ride the DMA queue behind it and arrive in time.
                        first = False
                        pt = pp.tile([P, 5, NG], f32, tag="params")
                        nc.sync.dma_start(out=pt, in_=params[:, :, :])
                        eyet = pp.tile([P, P], f32, tag="eye")
                        nc.sync.dma_start(out=eyet, in_=eye[:, :])
                    for g in range(NG):
                        W = gw = GW[g]
                        t = "" if gw == P else "L"
                        pin_t = (pin if gw == P else pinl).tile(
                            [W, SR], f32, tag="pin" + t)
                        for j in range(R2):
                            nc.tensor.transpose(
                                pin_t[:, j * P:(j + 1) * P],
                                zt_t[:, j, GO[g]:GO[g] + W],
                                eyet,
                            )
                        sb1 = s1p.tile([W, SR], f32, tag="sb1" + t)
                        nc.scalar.activation(
                            sb1, pin_t, ident,
                            bias=pt[:W, 1, g:g + 1], scale=pt[:W, 0, g:g + 1],
                        )
                        sb2 = s2p.tile([W, SR], bf16, tag="sb2" + t)
                        nc.vector.tensor_scalar(
                            sb2, sb1, pt[:W, 2, g:g + 1], pt[:W, 3, g:g + 1],
                            add, amax
                        )
                        dst = (outt[:, g, off:off + SR] if gw == P
                               else outl[:, off:off + SR])
                        nc.vector.tensor_scalar_min(dst, sb2, pt[:W, 4, g:g + 1])
                # Dispatched on the otherwise-idle Pool SEQ (SWDGE) so its
                # semaphore wait never blocks a compute engine's sequencer.
                rs = slice(base * SR, (base + gsz) * SR)
                nc.gpsimd.dma_start(out=ot[:, :, rs], in_=outt)
                nc.gpsimd.dma_start(out=out1[:, rs], in_=outl)
    nc.compile()
    return nc


def _host_params(eta_np):
    """Per-table-row params (f32, reference rounding).

    Returns (s, nx0s, y0, lo, hi, const, bad) where nx0s = -(s*x0) so the
    device computes (z*s + nx0s) + y0 == (z - x0)*s + y0 to ~1 ulp
    (bitwise for the s=1 and s=0 rows, the only ones whose outputs reach
    the rel-err denominator floor). `const` marks rows with y1 == y0 and
    x1 > x0: output is the constant y0 for every z (host fills it).
    """
    eta_np = eta_np.astype(np.float32)
    y0 = eta_np[:, 0]
    y1 = eta_np[:, 1]
    x0 = eta_np[:, 2]
    x1 = eta_np[:, 3]
    dx = x1 - x0                                   # f32, as in reference
    with np.errstate(divide="ignore", invalid="ignore"):
        s = (y1 - y0) / dx                         # f32, bitwise matches XLA
    nx0s = -(s * x0)                               # f32 product, then exact negate
    lo = np.minimum(y0, y1)
    hi = np.maximum(y0, y1)
    bad = ~((dx > 0) & np.isfinite(s))
    const = (y1 == y0) & ~bad
    z32 = np.float32(0)
    return (np.where(bad, z32, s), np.where(bad, z32, nx0s),
            np.where(bad, z32, y0), np.where(bad, z32, lo),
            np.where(bad, z32, hi), const, bad)


def _param_pack(par_cols):
    """list of 5 [F2] arrays -> [P, 5, NG] with (p, j, g) = param_j[g*P + p]
    (zero-padded where g*P + p >= F2)."""
    stack = np.zeros((5, NG * P), dtype=np.float32)
    stack[:, :F2] = np.stack(par_cols)
    return np.ascontiguousarray(
        stack.reshape(5, NG, P).transpose(2, 0, 1)   # [P, 5, NG]
    )


def _shard_plan(Mask, eta):
    """Per-batch device-column plan.

    Returns (kcols[b], extra[b], const_bf, bad_bf, par_bf) where kcols are
    the <=F2 source columns evaluated on device, extra are overflow columns
    (host-evaluated; empty for the standard Mask), const_bf marks
    constant-output columns and par_bf is the 5 per-column param arrays.
    """
    s_r, nx_r, y0_r, lo_r, hi_r, const_r, bad_r = _host_params(eta)
    mask_i = Mask.astype(np.int64)
    par_bf = [a[mask_i] for a in (s_r, nx_r, y0_r, lo_r, hi_r)]   # each [B, F]
    const_bf = const_r[mask_i]
    bad_bf = bad_r[mask_i]
    kcols, extra = [], []
    for b in range(B):
        (k,) = np.nonzero(~const_bf[b])
        kcols.append(k[:F2])
        extra.append(k[F2:])
    return kcols, extra, const_bf, bad_bf, par_bf


def make_in_maps(z, Mask, eta):
    """Shard z over cores and build per-core input maps.

    Returns (in_maps, plan); plan = (kcols, extra, const_bf, bad_bf).
    """
    kcols, extra, const_bf, bad_bf, par_bf = _shard_plan(Mask, eta)
    eye = np.eye(P, dtype=np.float32)

    params_b = []
    for b in range(B):
        k = kcols[b]
        cols = []
        for a in par_bf:
            c = np.zeros(F2, dtype=np.float32)
            c[:k.size] = a[b][k]
            cols.append(c)
        params_b.append(_param_pack(cols))

    in_maps = []
    for core in range(NCORES):
        b, nh = core // 2, core % 2
        zs = z[b, nh * NH:(nh + 1) * NH].reshape(ROWS, F)
        k = kcols[b]
        z2 = np.zeros((ROWS, F2), dtype=np.float32)
        z2[:, :k.size] = zs[:, k]
        in_maps.append({"z": z2, "params": params_b[b], "eye": eye})
    return in_maps, (kcols, extra, const_bf, bad_bf)


def _unshard(res_core):
    """Device out [W, ROWS'] bf16 -> [ROWS, W] f32.

    Column r' = tr*SR + j*128 + p holds row tr*SR + p*R2 + j.
    """
    u16 = np.asarray(res_core).view(np.uint16)       # [W, ROWS']
    W = u16.shape[0]
    u16 = u16.reshape(W, NTR, R2, P)                 # [f, tr, j, p]
    u16 = u16.transpose(1, 3, 2, 0)                  # [tr, p, j, f]
    u32 = u16.astype(np.uint32) << np.uint32(16)
    return u32.view(np.float32).reshape(ROWS, W)


def _ref_cols(z3, eta_g):
    """Exact reference math for selected columns: z3 [R, ncols],
    eta_g [ncols, 4] -> [R, ncols] f32."""
    y0 = eta_g[:, 0]
    y1 = eta_g[:, 1]
    x0 = eta_g[:, 2]
    x1 = eta_g[:, 3]
    with np.errstate(divide="ignore", invalid="ignore"):
        lin = y0 + (y1 - y0) / (x1 - x0) * (z3 - x0)
    return np.where(z3 < x0, y0, np.where(z3 <= x1, lin, y1)).astype(np.float32)


def kernel(z, Mask, eta_fault):
    z = np.ascontiguousarray(np.asarray(z, dtype=np.float32))
    Mask = np.asarray(Mask)
    eta = np.asarray(eta_fault, dtype=np.float32)

    if "nc" not in _nc_cache:
        _nc_cache["nc"] = _build_nc()
    nc = _nc_cache["nc"]

    in_maps, (kcols, extra, const_bf, bad_bf) = make_in_maps(z, Mask, eta)
    mask_i = Mask.astype(np.int64)
    eta_bf = eta[mask_i]                             # [B, F, 4] f32

    res = run_bass_kernel_spmd(nc, in_maps, list(range(NCORES)))

    # Constant-output columns: fill y0 (broadcast over all rows).
    y0_bf = eta_bf[:, :, 0]                          # [B, F]
    out = np.empty((B, N, M, F), dtype=np.float32)
    for b in range(B):
        cb = const_bf[b]
        out[b, :, :, cb] = y0_bf[b, cb][:, None, None]

    NP0 = (NG - 1) * P                               # cols in out0
    for core in range(NCORES):
        b, nh = core // 2, core % 2
        k = kcols[b]
        view = out[b, nh * NH:(nh + 1) * NH].reshape(ROWS, F)
        arr0 = _unshard(res.results[core]["out0"])   # [ROWS, 768]
        view[:, k[:NP0]] = arr0[:, :min(k.size, NP0)]
        if k.size > NP0:
            arr1 = _unshard(res.results[core]["out1"])  # [ROWS, 96]
            view[:, k[NP0:]] = arr1[:, :k.size - NP0]
        if extra[b].size:                            # F2 overflow: host math
            zs = z[b, nh * NH:(nh + 1) * NH].reshape(ROWS, F)
            view[:, extra[b]] = _ref_cols(zs[:, extra[b]], eta_bf[b, extra[b]])

    # Host patch for degenerate rows (never triggers with the standard table).
    if bad_bf.any():
        for b in range(B):
            (fbad,) = np.nonzero(bad_bf[b])
            if fbad.size == 0:
                continue
            zb = z[b][:, :, fbad]                    # [N, M, nbad]
            out[b][:, :, fbad] = _ref_cols(
                zb.reshape(-1, fbad.size), eta_bf[b, fbad]
            ).reshape(N, M, fbad.size)

    return out


# revision 22
# speedup vs baseline: 2.1901x; 1.4303x over previous
"""Trainium2 Bass kernel for nn_ClippedReLU (piecewise-linear clip).

Reference semantics:
    eta = eta_fault[Mask]                 # [B, F, 4] rows (y0, y1, x0, x1)
    s   = (y1-y0)/(x1-x0)
    lin = y0 + s*(z - x0)
    out = where(z < x0, y0, where(z <= x1, lin, y1))

For rows with x1 > x0 this equals clamp(lin, min(y0,y1), max(y0,y1)).
The device evaluates lin as (z*s - x0*s) + y0; the two orders agree to
~1 ulp, and agree bitwise for the only rows whose outputs approach the
1e-6 rel-err denominator floor (the identity row s=1, x0*s=-2, where the
reference's fl(z+2)-2 snapping is reproduced exactly, and s=0 rows).
The result is rounded once to bf16 (rel err <= 2^-9), halving the
output stream.

Byte-diet on top of data-parallel sharding:
  * out stream is bf16 (host upconverts with a u16<<16 shift).
  * columns whose table row has y1 == y0 (the standard table's row 5)
    produce the constant y0 for every z: the reference's lin is
    y0 + 0*(z-x0) = y0 and both where-branches also give y0. Those
    columns are dropped from the device entirely (no z read, no out
    write); the host fills the constant. Per-core column count is
    compacted+padded to F2=896 (the key(0) Mask gives <=860; if a
    different Mask ever exceeds F2, the excess columns are evaluated
    on the host with exact reference math).
Per core that leaves 8192*896*4 B in + 8192*896*2 B out ~= 42 MiB
~= 123 us at the 358 GB/s DMA roofline (vs 187 us for the f32
full-width streams).

Sharding: core i takes b = i//2 and N-half i%2 (a contiguous
[8, 1024, 1024] block = [8192, 1024] rows).

Device pipeline per (row-supertile tr of 512 rows, f-block g of 128):
  1. per tr: one 1.75 MiB DMA in (SP HWDGE); partition p holds the 4
     consecutive rows tr*512 + p*4 .. +3 -> one 14 KiB descriptor per
     partition (cheap dispatch).
  2. PE: 4 transposes [128,128] -> psum [f=128, 512] (free idx j*128+p
     holds row p*4+j)
  3. ACT: activation Identity, scale=s[p], bias=-x0*s[p]  (psum->sbuf f32)
  4. DVE: tensor_scalar (+y0[p], max lo[p]) f32->bf16   (2x SBUF mode)
  5. DVE: tensor_scalar (min hi[p])         bf16->bf16  (4x mode)
  6. per tr: ONE strided DMA out (Pool SWDGE, 1 KiB lines) covering all
     7 f-blocks, so no compute sequencer ever stalls on a DMA wait.
Host un-compacts/un-transposes and upconverts bf16->f32.

Degenerate rows (x1 <= x0 or non-finite slope; impossible with the
standard table) are patched on the host with exact reference semantics.
"""

import numpy as np

import concourse.bacc as bacc
import concourse.mybir as mybir
from concourse.tile import TileContext
from concourse.bass_utils import run_bass_kernel_spmd

B, N, M, F = 4, 16, 1024, 1024
NCORES = 8
NH = N // 2                # N-rows per core
ROWS = NH * M              # 8192 flattened rows per core
P = 128                    # SBUF partitions
R2 = 4                     # consecutive DRAM rows per partition per supertile
SR = R2 * P                # 512 rows per supertile
NTR = ROWS // SR           # 16 row-supertiles
NG = 7                     # f-blocks on device (last one partial)
PL = 96                    # width of the last (partial) f-block
F2 = (NG - 1) * P + PL     # 864 compacted+padded columns on device
GW = [P] * (NG - 1) + [PL]  # per-block widths
GO = [g * P for g in range(NG)]  # per-block column offsets

_nc_cache = {}


def _build_nc():
    f32 = mybir.dt.float32
    f16 = mybir.dt.float16
    bf16 = mybir.dt.bfloat16
    nc = bacc.Bacc("TRN2", debug=False)
    z = nc.dram_tensor("z", [ROWS, F2], f16, kind="ExternalInput")
    params = nc.dram_tensor("params", [P, 5, NG], f32, kind="ExternalInput")
    eye = nc.dram_tensor("eye", [P, P], f16, kind="ExternalInput")
    # out0/out1: device cols [0,768) / [768,864); col f at row'
    # r' = tr*SR + j*128 + p holds input row tr*SR + p*R2 + j.
    NG0 = NG - 1
    out0 = nc.dram_tensor("out0", [NG0 * P, ROWS], bf16, kind="ExternalOutput")
    out1 = nc.dram_tensor("out1", [PL, ROWS], bf16, kind="ExternalOutput")

    # [tr, p, j, f]: row = tr*SR + p*R2 + j (R2 consecutive rows/partition)
    zt = z.rearrange("(tr p j) f -> tr p j f", p=P, j=R2)
    # [p, g, r]: out0[g*P + p, r] -- sliced per out-DMA group
    ot = out0.rearrange("(g p) r -> p g r", p=P)

    add = mybir.AluOpType.add
    amax = mybir.AluOpType.max
    ident = mybir.ActivationFunctionType.Identity

    with TileContext(nc) as tc:
        # Out-DMA groups: big early groups defer output traffic so the
        # input stream owns the (FIFO) DMA engines until it finishes;
        # per-supertile singles at the end keep the drain tail short.
        groups = [(0, 4), (4, 4), (8, 4), (12, 1), (13, 1), (14, 1), (15, 1)]
        with (
            tc.tile_pool(name="pp", bufs=1) as pp,
            tc.tile_pool(name="io", bufs=5) as io,
            tc.tile_pool(name="s1", bufs=4) as s1p,
            tc.tile_pool(name="s2", bufs=4) as s2p,
            tc.tile_pool(name="og", bufs=2) as ogp,
            tc.tile_pool(name="os", bufs=4) as osp,
            tc.tile_pool(name="pin", bufs=4, space="PSUM") as pin,
            tc.tile_pool(name="pinl", bufs=3, space="PSUM") as pinl,
        ):
            first = True
            pt = eyet = None
            for base, gsz in groups:
                pool = ogp if gsz > 1 else osp
                sfx = "g" if gsz > 1 else "s"
                outt = pool.tile([P, NG0, gsz * SR], bf16, tag="o" + sfx)
                outl = pool.tile([PL, gsz * SR], bf16, tag="l" + sfx)
                for tr in range(base, base + gsz):
                    off = (tr - base) * SR
                    zt_t = io.tile([P, R2, F2], f16, tag="z")
                    nc.sync.dma_start(out=zt_t, in_=zt[tr])
                    if first:
                        # After the first (critical-path) z dispatch: these
                        # ride the DMA queue behind it and arrive in time.
                        first = False
                        pt = pp.tile([P, 5, NG], f32, tag="params")
                        nc.sync.dma_start(out=pt, in_=params[:, :, :])
                        eyet = pp.tile([P, P], f16, tag="eye")
                        nc.sync.dma_start(out=eyet, in_=eye[:, :])
                    for g in range(NG):
                        W = gw = GW[g]
                        t = "" if gw == P else "L"
                        pin_t = (pin if gw == P else pinl).tile(
                            [W, SR], f16, tag="pin" + t)
                        for j in range(R2):
                            nc.tensor.transpose(
                                pin_t[:, j * P:(j + 1) * P],
                                zt_t[:, j, GO[g]:GO[g] + W],
                                eyet,
                            )
                        sb1 = s1p.tile([W, SR], f32, tag="sb1" + t)
                        nc.scalar.activation(
                            sb1, pin_t, ident,
                            bias=pt[:W, 1, g:g + 1], scale=pt[:W, 0, g:g + 1],
                        )
                        sb2 = s2p.tile([W, SR], bf16, tag="sb2" + t)
                        nc.vector.tensor_scalar(
                            sb2, sb1, pt[:W, 2, g:g + 1], pt[:W, 3, g:g + 1],
                            add, amax
                        )
                        dst = (outt[:, g, off:off + SR] if gw == P
                               else outl[:, off:off + SR])
                        nc.vector.tensor_scalar_min(dst, sb2, pt[:W, 4, g:g + 1])
                # Dispatched on the otherwise-idle Pool SEQ (SWDGE) so its
                # semaphore wait never blocks a compute engine's sequencer.
                rs = slice(base * SR, (base + gsz) * SR)
                nc.gpsimd.dma_start(out=ot[:, :, rs], in_=outt)
                nc.gpsimd.dma_start(out=out1[:, rs], in_=outl)
    nc.compile()
    return nc


def _host_params(eta_np):
    """Per-table-row params (f32, reference rounding).

    Returns (s, nx0s, y0, lo, hi, const, bad) where nx0s = -(s*x0) so the
    device computes (z*s + nx0s) + y0 == (z - x0)*s + y0 to ~1 ulp
    (bitwise for the s=1 and s=0 rows, the only ones whose outputs reach
    the rel-err denominator floor). `const` marks rows with y1 == y0 and
    x1 > x0: output is the constant y0 for every z (host fills it).
    """
    eta_np = eta_np.astype(np.float32)
    y0 = eta_np[:, 0]
    y1 = eta_np[:, 1]
    x0 = eta_np[:, 2]
    x1 = eta_np[:, 3]
    dx = x1 - x0                                   # f32, as in reference
    with np.errstate(divide="ignore", invalid="ignore"):
        s = (y1 - y0) / dx                         # f32, bitwise matches XLA
    nx0s = -(s * x0)                               # f32 product, then exact negate
    lo = np.minimum(y0, y1)
    hi = np.maximum(y0, y1)
    bad = ~((dx > 0) & np.isfinite(s))
    const = (y1 == y0) & ~bad
    z32 = np.float32(0)
    return (np.where(bad, z32, s), np.where(bad, z32, nx0s),
            np.where(bad, z32, y0), np.where(bad, z32, lo),
            np.where(bad, z32, hi), const, bad)


def _param_pack(par_cols):
    """list of 5 [F2] arrays -> [P, 5, NG] with (p, j, g) = param_j[g*P + p]
    (zero-padded where g*P + p >= F2)."""
    stack = np.zeros((5, NG * P), dtype=np.float32)
    stack[:, :F2] = np.stack(par_cols)
    return np.ascontiguousarray(
        stack.reshape(5, NG, P).transpose(2, 0, 1)   # [P, 5, NG]
    )


def _shard_plan(Mask, eta):
    """Per-batch device-column plan.

    Returns (kcols[b], extra[b], const_bf, bad_bf, par_bf) where kcols are
    the <=F2 source columns evaluated on device, extra are overflow columns
    (host-evaluated; empty for the standard Mask), const_bf marks
    constant-output columns and par_bf is the 5 per-column param arrays.
    """
    s_r, nx_r, y0_r, lo_r, hi_r, const_r, bad_r = _host_params(eta)
    mask_i = Mask.astype(np.int64)
    par_bf = [a[mask_i] for a in (s_r, nx_r, y0_r, lo_r, hi_r)]   # each [B, F]
    const_bf = const_r[mask_i]
    bad_bf = bad_r[mask_i]
    kcols, extra = [], []
    for b in range(B):
        (k,) = np.nonzero(~const_bf[b])
        kcols.append(k[:F2])
        extra.append(k[F2:])
    return kcols, extra, const_bf, bad_bf, par_bf


def make_in_maps(z, Mask, eta):
    """Shard z over cores and build per-core input maps.

    Returns (in_maps, plan); plan = (kcols, extra, const_bf, bad_bf).
    """
    kcols, extra, const_bf, bad_bf, par_bf = _shard_plan(Mask, eta)
    eye = np.eye(P, dtype=np.float16)

    params_b = []
    for b in range(B):
        k = kcols[b]
        cols = []
        for a in par_bf:
            c = np.zeros(F2, dtype=np.float32)
            c[:k.size] = a[b][k]
            cols.append(c)
        params_b.append(_param_pack(cols))

    in_maps = []
    for core in range(NCORES):
        b, nh = core // 2, core % 2
        zs = z[b, nh * NH:(nh + 1) * NH].reshape(ROWS, F)
        k = kcols[b]
        z2 = np.zeros((ROWS, F2), dtype=np.float16)
        z2[:, :k.size] = zs[:, k].astype(np.float16)
        in_maps.append({"z": z2, "params": params_b[b], "eye": eye})
    return in_maps, (kcols, extra, const_bf, bad_bf)


def _unshard(res_core):
    """Device out [W, ROWS'] bf16 -> [ROWS, W] f32.

    Column r' = tr*SR + j*128 + p holds row tr*SR + p*R2 + j.
    """
    u16 = np.asarray(res_core).view(np.uint16)       # [W, ROWS']
    W = u16.shape[0]
    u16 = u16.reshape(W, NTR, R2, P)                 # [f, tr, j, p]
    u16 = u16.transpose(1, 3, 2, 0)                  # [tr, p, j, f]
    u32 = u16.astype(np.uint32) << np.uint32(16)
    return u32.view(np.float32).reshape(ROWS, W)


_HALF_ULP16 = 2.0 ** -11     # fp16 relative half-ulp (normals)


def _risk_bands(eta_np):
    """Per-table-row z-band (zlo, zhi) where the device's fp16-input error
    could exceed 10% of the 2e-2 rel-err gate (denominator floored at 1e-6);
    elements inside get exact host recomputation.

    The device error on out = clamp((z-x0)*s + y0, lo, hi) from rounding z
    to fp16 is <= s * 2^-11 * |z| (+ f32 quantization slop of the affine
    adds). It only matters where |out| is small, i.e. around the zero
    crossing zstar = x0 - y0/s. The band half-width w solves
    w = err(|zstar|+w) / (0.002 * s) -- a contraction since
    2^-11/0.002 * s*d(zmax)/dw < 0.25 -- iterated to convergence. Rows with
    s == 0 (const) or degenerate x-range (bad) have no band (handled
    elsewhere).
    """
    eta64 = eta_np.astype(np.float64)
    nrow = eta64.shape[0]
    zlo = np.full(nrow, np.inf)
    zhi = np.full(nrow, -np.inf)
    for r in range(nrow):
        y0, y1, x0, x1 = eta64[r]
        dx32 = np.float32(eta_np[r, 3] - eta_np[r, 2])
        s32 = np.float32(eta_np[r, 1] - eta_np[r, 0]) / dx32 if dx32 != 0 else np.float32(np.nan)
        if not (dx32 > 0 and np.isfinite(s32)) or s32 == 0:
            continue                                 # bad or const row
        s = float(s32)
        nx0s = s * x0
        zstar = x0 - y0 / s
        w = 0.01
        for _ in range(6):
            zmax = abs(zstar) + w
            slop = 2.2 * 2.0 ** -24 * max(s * zmax, abs(nx0s), abs(y0), 1e-30)
            err = s * _HALF_ULP16 * zmax + slop
            w = err / 0.002 / s
        cx0 = 2 * (_HALF_ULP16 * abs(x0) + 1e-6)
        cx1 = 2 * (_HALF_ULP16 * abs(x1) + 1e-6)
        lo = max(zstar - w, x0 - cx0)
        hi = min(zstar + w, x1 + cx1)
        if hi > lo:
            zlo[r], zhi[r] = lo, hi
    return zlo, zhi


def _ref_cols(z3, eta_g):
    """Exact reference math for selected columns: z3 [R, ncols],
    eta_g [ncols, 4] -> [R, ncols] f32."""
    y0 = eta_g[:, 0]
    y1 = eta_g[:, 1]
    x0 = eta_g[:, 2]
    x1 = eta_g[:, 3]
    with np.errstate(divide="ignore", invalid="ignore"):
        lin = y0 + (y1 - y0) / (x1 - x0) * (z3 - x0)
    return np.where(z3 < x0, y0, np.where(z3 <= x1, lin, y1)).astype(np.float32)


def kernel(z, Mask, eta_fault):
    z = np.ascontiguousarray(np.asarray(z, dtype=np.float32))
    Mask = np.asarray(Mask)
    eta = np.asarray(eta_fault, dtype=np.float32)

    if "nc" not in _nc_cache:
        _nc_cache["nc"] = _build_nc()
    nc = _nc_cache["nc"]

    in_maps, (kcols, extra, const_bf, bad_bf) = make_in_maps(z, Mask, eta)
    mask_i = Mask.astype(np.int64)
    eta_bf = eta[mask_i]                             # [B, F, 4] f32

    res = run_bass_kernel_spmd(nc, in_maps, list(range(NCORES)))

    # Constant-output columns: fill y0 (broadcast over all rows).
    y0_bf = eta_bf[:, :, 0]                          # [B, F]
    out = np.empty((B, N, M, F), dtype=np.float32)
    for b in range(B):
        cb = const_bf[b]
        out[b, :, :, cb] = y0_bf[b, cb][:, None, None]

    NP0 = (NG - 1) * P                               # cols in out0
    for core in range(NCORES):
        b, nh = core // 2, core % 2
        k = kcols[b]
        view = out[b, nh * NH:(nh + 1) * NH].reshape(ROWS, F)
        arr0 = _unshard(res.results[core]["out0"])   # [ROWS, 768]
        view[:, k[:NP0]] = arr0[:, :min(k.size, NP0)]
        if k.size > NP0:
            arr1 = _unshard(res.results[core]["out1"])  # [ROWS, 96]
            view[:, k[NP0:]] = arr1[:, :k.size - NP0]
        if extra[b].size:                            # F2 overflow: host math
            zs = z[b, nh * NH:(nh + 1) * NH].reshape(ROWS, F)
            view[:, extra[b]] = _ref_cols(zs[:, extra[b]], eta_bf[b, extra[b]])

    # Exact host recomputation where the fp16 z rounding could matter:
    # elements whose z falls in the per-column risk band around the clip
    # knee (|out| small there). ~1.6% of elements for the standard table.
    zlo_r, zhi_r = _risk_bands(eta)
    zlo_bf = zlo_r[mask_i]                           # [B, F]
    zhi_bf = zhi_r[mask_i]
    risky = (z > zlo_bf[:, None, None, :]) & (z < zhi_bf[:, None, None, :])
    risky &= ~const_bf[:, None, None, :]
    if bad_bf.any():
        risky &= ~bad_bf[:, None, None, :]
    if risky.any():
        bi, _, _, fi = np.nonzero(risky)
        zv = z[risky]                                # f32, C-order like nonzero
        eg = eta_bf[bi, fi]                          # [K, 4] f32
        y0 = eg[:, 0]
        y1 = eg[:, 1]
        x0 = eg[:, 2]
        x1 = eg[:, 3]
        with np.errstate(divide="ignore", invalid="ignore"):
            lin = y0 + (y1 - y0) / (x1 - x0) * (zv - x0)
        out[risky] = np.where(
            zv < x0, y0, np.where(zv <= x1, lin, y1)).astype(np.float32)

    # Host patch for degenerate rows (never triggers with the standard table).
    if bad_bf.any():
        for b in range(B):
            (fbad,) = np.nonzero(bad_bf[b])
            if fbad.size == 0:
                continue
            zb = z[b][:, :, fbad]                    # [N, M, nbad]
            out[b][:, :, fbad] = _ref_cols(
                zb.reshape(-1, fbad.size), eta_bf[b, fbad]
            ).reshape(N, M, fbad.size)

    return out


# revision 24
# speedup vs baseline: 2.2693x; 1.0362x over previous
"""Trainium2 Bass kernel for nn_ClippedReLU (piecewise-linear clip).

Reference semantics:
    eta = eta_fault[Mask]                 # [B, F, 4] rows (y0, y1, x0, x1)
    s   = (y1-y0)/(x1-x0)
    lin = y0 + s*(z - x0)
    out = where(z < x0, y0, where(z <= x1, lin, y1))

For rows with x1 > x0 this equals clamp(lin, min(y0,y1), max(y0,y1)).
The device evaluates lin as (z*s - x0*s) + y0; the two orders agree to
~1 ulp, and agree bitwise for the only rows whose outputs approach the
1e-6 rel-err denominator floor (the identity row s=1, x0*s=-2, where the
reference's fl(z+2)-2 snapping is reproduced exactly, and s=0 rows).
The result is rounded once to bf16 (rel err <= 2^-9), halving the
output stream.

Byte-diet on top of data-parallel sharding:
  * out stream is bf16 (host upconverts with a u16<<16 shift).
  * columns whose table row has y1 == y0 (the standard table's row 5)
    produce the constant y0 for every z: the reference's lin is
    y0 + 0*(z-x0) = y0 and both where-branches also give y0. Those
    columns are dropped from the device entirely (no z read, no out
    write); the host fills the constant. Per-core column count is
    compacted+padded to F2=896 (the key(0) Mask gives <=860; if a
    different Mask ever exceeds F2, the excess columns are evaluated
    on the host with exact reference math).
Per core that leaves 8192*896*4 B in + 8192*896*2 B out ~= 42 MiB
~= 123 us at the 358 GB/s DMA roofline (vs 187 us for the f32
full-width streams).

Sharding: core i takes b = i//2 and N-half i%2 (a contiguous
[8, 1024, 1024] block = [8192, 1024] rows).

Device pipeline per (row-supertile tr of 512 rows, f-block g of 128):
  1. per tr: one 1.75 MiB DMA in (SP HWDGE); partition p holds the 4
     consecutive rows tr*512 + p*4 .. +3 -> one 14 KiB descriptor per
     partition (cheap dispatch).
  2. PE: 4 transposes [128,128] -> psum [f=128, 512] (free idx j*128+p
     holds row p*4+j)
  3. ACT: activation Identity, scale=s[p], bias=-x0*s[p]  (psum->sbuf f32)
  4. DVE: tensor_scalar (+y0[p], max lo[p]) f32->bf16   (2x SBUF mode)
  5. DVE: tensor_scalar (min hi[p])         bf16->bf16  (4x mode)
  6. per tr: ONE strided DMA out (Pool SWDGE, 1 KiB lines) covering all
     7 f-blocks, so no compute sequencer ever stalls on a DMA wait.
Host un-compacts/un-transposes and upconverts bf16->f32.

Degenerate rows (x1 <= x0 or non-finite slope; impossible with the
standard table) are patched on the host with exact reference semantics.
"""

import numpy as np

import concourse.bacc as bacc
import concourse.mybir as mybir
from concourse.tile import TileContext
from concourse.bass_utils import run_bass_kernel_spmd

B, N, M, F = 4, 16, 1024, 1024
NCORES = 8
NH = N // 2                # N-rows per core
ROWS = NH * M              # 8192 flattened rows per core
P = 128                    # SBUF partitions
R2 = 4                     # consecutive DRAM rows per partition per supertile
SR = R2 * P                # 512 rows per supertile
NTR = ROWS // SR           # 16 row-supertiles
NG = 7                     # f-blocks on device (last one partial)
PL = 92                    # width of the last (partial) f-block
F2 = (NG - 1) * P + PL     # 864 compacted+padded columns on device
GW = [P] * (NG - 1) + [PL]  # per-block widths
GO = [g * P for g in range(NG)]  # per-block column offsets

_nc_cache = {}


def _build_nc():
    f32 = mybir.dt.float32
    f16 = mybir.dt.float16
    bf16 = mybir.dt.bfloat16
    nc = bacc.Bacc("TRN2", debug=False)
    z = nc.dram_tensor("z", [ROWS, F2], f16, kind="ExternalInput")
    params = nc.dram_tensor("params", [P, 5, NG], f32, kind="ExternalInput")
    eye = nc.dram_tensor("eye", [P, P], f16, kind="ExternalInput")
    # out0/out1: device cols [0,768) / [768,864); col f at row'
    # r' = tr*SR + j*128 + p holds input row tr*SR + p*R2 + j.
    NG0 = NG - 1
    out0 = nc.dram_tensor("out0", [NG0 * P, ROWS], bf16, kind="ExternalOutput")
    out1 = nc.dram_tensor("out1", [PL, ROWS], bf16, kind="ExternalOutput")

    # [tr, p, j, f]: row = tr*SR + p*R2 + j (R2 consecutive rows/partition)
    zt = z.rearrange("(tr p j) f -> tr p j f", p=P, j=R2)
    # [p, g, r]: out0[g*P + p, r] -- sliced per out-DMA group
    ot = out0.rearrange("(g p) r -> p g r", p=P)

    add = mybir.AluOpType.add
    amax = mybir.AluOpType.max
    ident = mybir.ActivationFunctionType.Identity

    with TileContext(nc) as tc:
        # Out-DMA groups: big early groups defer output traffic so the
        # input stream owns the (FIFO) DMA engines until it finishes;
        # per-supertile singles at the end keep the drain tail short.
        groups = [(0, 4), (4, 4), (8, 4), (12, 1), (13, 1), (14, 1), (15, 1)]
        with (
            tc.tile_pool(name="pp", bufs=1) as pp,
            tc.tile_pool(name="io", bufs=8) as io,
            tc.tile_pool(name="s1", bufs=4) as s1p,
            tc.tile_pool(name="s2", bufs=4) as s2p,
            tc.tile_pool(name="og", bufs=2) as ogp,
            tc.tile_pool(name="os", bufs=4) as osp,
            tc.tile_pool(name="pin", bufs=4, space="PSUM") as pin,
            tc.tile_pool(name="pinl", bufs=3, space="PSUM") as pinl,
        ):
            first = True
            pt = eyet = None
            for base, gsz in groups:
                pool = ogp if gsz > 1 else osp
                sfx = "g" if gsz > 1 else "s"
                outt = pool.tile([P, NG0, gsz * SR], bf16, tag="o" + sfx)
                outl = pool.tile([PL, gsz * SR], bf16, tag="l" + sfx)
                for tr in range(base, base + gsz):
                    off = (tr - base) * SR
                    zt_t = io.tile([P, R2, F2], f16, tag="z")
                    nc.sync.dma_start(out=zt_t, in_=zt[tr])
                    if first:
                        # After the first (critical-path) z dispatch: these
                        # ride the DMA queue behind it and arrive in time.
                        first = False
                        pt = pp.tile([P, 5, NG], f32, tag="params")
                        nc.sync.dma_start(out=pt, in_=params[:, :, :])
                        eyet = pp.tile([P, P], f16, tag="eye")
                        nc.sync.dma_start(out=eyet, in_=eye[:, :])
                    for g in range(NG):
                        W = gw = GW[g]
                        t = "" if gw == P else "L"
                        pin_t = (pin if gw == P else pinl).tile(
                            [W, SR], f16, tag="pin" + t)
                        for j in range(R2):
                            nc.tensor.transpose(
                                pin_t[:, j * P:(j + 1) * P],
                                zt_t[:, j, GO[g]:GO[g] + W],
                                eyet,
                            )
                        sb1 = s1p.tile([W, SR], f32, tag="sb1" + t)
                        nc.scalar.activation(
                            sb1, pin_t, ident,
                            bias=pt[:W, 1, g:g + 1], scale=pt[:W, 0, g:g + 1],
                        )
                        sb2 = s2p.tile([W, SR], bf16, tag="sb2" + t)
                        nc.vector.tensor_scalar(
                            sb2, sb1, pt[:W, 2, g:g + 1], pt[:W, 3, g:g + 1],
                            add, amax
                        )
                        dst = (outt[:, g, off:off + SR] if gw == P
                               else outl[:, off:off + SR])
                        nc.vector.tensor_scalar_min(dst, sb2, pt[:W, 4, g:g + 1])
                # Dispatched on the otherwise-idle Pool SEQ (SWDGE) so its
                # semaphore wait never blocks a compute engine's sequencer.
                rs = slice(base * SR, (base + gsz) * SR)
                nc.gpsimd.dma_start(out=ot[:, :, rs], in_=outt)
                nc.gpsimd.dma_start(out=out1[:, rs], in_=outl)
    nc.compile()
    return nc


def _host_params(eta_np):
    """Per-table-row params (f32, reference rounding).

    Returns (s, nx0s, y0, lo, hi, const, bad) where nx0s = -(s*x0) so the
    device computes (z*s + nx0s) + y0 == (z - x0)*s + y0 to ~1 ulp
    (bitwise for the s=1 and s=0 rows, the only ones whose outputs reach
    the rel-err denominator floor). `const` marks rows with y1 == y0 and
    x1 > x0: output is the constant y0 for every z (host fills it).
    """
    eta_np = eta_np.astype(np.float32)
    y0 = eta_np[:, 0]
    y1 = eta_np[:, 1]
    x0 = eta_np[:, 2]
    x1 = eta_np[:, 3]
    dx = x1 - x0                                   # f32, as in reference
    with np.errstate(divide="ignore", invalid="ignore"):
        s = (y1 - y0) / dx                         # f32, bitwise matches XLA
    nx0s = -(s * x0)                               # f32 product, then exact negate
    lo = np.minimum(y0, y1)
    hi = np.maximum(y0, y1)
    bad = ~((dx > 0) & np.isfinite(s))
    const = (y1 == y0) & ~bad
    z32 = np.float32(0)
    return (np.where(bad, z32, s), np.where(bad, z32, nx0s),
            np.where(bad, z32, y0), np.where(bad, z32, lo),
            np.where(bad, z32, hi), const, bad)


def _param_pack(par_cols):
    """list of 5 [F2] arrays -> [P, 5, NG] with (p, j, g) = param_j[g*P + p]
    (zero-padded where g*P + p >= F2)."""
    stack = np.zeros((5, NG * P), dtype=np.float32)
    stack[:, :F2] = np.stack(par_cols)
    return np.ascontiguousarray(
        stack.reshape(5, NG, P).transpose(2, 0, 1)   # [P, 5, NG]
    )


def _shard_plan(Mask, eta):
    """Per-batch device-column plan.

    Returns (kcols[b], extra[b], const_bf, bad_bf, par_bf) where kcols are
    the <=F2 source columns evaluated on device, extra are overflow columns
    (host-evaluated; empty for the standard Mask), const_bf marks
    constant-output columns and par_bf is the 5 per-column param arrays.
    """
    s_r, nx_r, y0_r, lo_r, hi_r, const_r, bad_r = _host_params(eta)
    mask_i = Mask.astype(np.int64)
    par_bf = [a[mask_i] for a in (s_r, nx_r, y0_r, lo_r, hi_r)]   # each [B, F]
    const_bf = const_r[mask_i]
    bad_bf = bad_r[mask_i]
    kcols, extra = [], []
    for b in range(B):
        (k,) = np.nonzero(~const_bf[b])
        kcols.append(k[:F2])
        extra.append(k[F2:])
    return kcols, extra, const_bf, bad_bf, par_bf


def make_in_maps(z, Mask, eta):
    """Shard z over cores and build per-core input maps.

    Returns (in_maps, plan); plan = (kcols, extra, const_bf, bad_bf).
    """
    kcols, extra, const_bf, bad_bf, par_bf = _shard_plan(Mask, eta)
    eye = np.eye(P, dtype=np.float16)

    params_b = []
    for b in range(B):
        k = kcols[b]
        cols = []
        for a in par_bf:
            c = np.zeros(F2, dtype=np.float32)
            c[:k.size] = a[b][k]
            cols.append(c)
        params_b.append(_param_pack(cols))

    in_maps = []
    for core in range(NCORES):
        b, nh = core // 2, core % 2
        zs = z[b, nh * NH:(nh + 1) * NH].reshape(ROWS, F)
        k = kcols[b]
        z2 = np.zeros((ROWS, F2), dtype=np.float16)
        z2[:, :k.size] = zs[:, k].astype(np.float16)
        in_maps.append({"z": z2, "params": params_b[b], "eye": eye})
    return in_maps, (kcols, extra, const_bf, bad_bf)


def _unshard(res_core):
    """Device out [W, ROWS'] bf16 -> [ROWS, W] f32.

    Column r' = tr*SR + j*128 + p holds row tr*SR + p*R2 + j.
    """
    u16 = np.asarray(res_core).view(np.uint16)       # [W, ROWS']
    W = u16.shape[0]
    u16 = u16.reshape(W, NTR, R2, P)                 # [f, tr, j, p]
    u16 = u16.transpose(1, 3, 2, 0)                  # [tr, p, j, f]
    u32 = u16.astype(np.uint32) << np.uint32(16)
    return u32.view(np.float32).reshape(ROWS, W)


_HALF_ULP16 = 2.0 ** -11     # fp16 relative half-ulp (normals)


def _risk_bands(eta_np):
    """Per-table-row z-band (zlo, zhi) where the device's fp16-input error
    could exceed 10% of the 2e-2 rel-err gate (denominator floored at 1e-6);
    elements inside get exact host recomputation.

    The device error on out = clamp((z-x0)*s + y0, lo, hi) from rounding z
    to fp16 is <= s * 2^-11 * |z| (+ f32 quantization slop of the affine
    adds). It only matters where |out| is small, i.e. around the zero
    crossing zstar = x0 - y0/s. The band half-width w solves
    w = err(|zstar|+w) / (0.002 * s) -- a contraction since
    2^-11/0.002 * s*d(zmax)/dw < 0.25 -- iterated to convergence. Rows with
    s == 0 (const) or degenerate x-range (bad) have no band (handled
    elsewhere).
    """
    eta64 = eta_np.astype(np.float64)
    nrow = eta64.shape[0]
    zlo = np.full(nrow, np.inf)
    zhi = np.full(nrow, -np.inf)
    for r in range(nrow):
        y0, y1, x0, x1 = eta64[r]
        dx32 = np.float32(eta_np[r, 3] - eta_np[r, 2])
        s32 = np.float32(eta_np[r, 1] - eta_np[r, 0]) / dx32 if dx32 != 0 else np.float32(np.nan)
        if not (dx32 > 0 and np.isfinite(s32)) or s32 == 0:
            continue                                 # bad or const row
        s = float(s32)
        nx0s = s * x0
        zstar = x0 - y0 / s
        w = 0.01
        for _ in range(6):
            zmax = abs(zstar) + w
            slop = 2.2 * 2.0 ** -24 * max(s * zmax, abs(nx0s), abs(y0), 1e-30)
            err = s * _HALF_ULP16 * zmax + slop
            w = err / 0.002 / s
        cx0 = 2 * (_HALF_ULP16 * abs(x0) + 1e-6)
        cx1 = 2 * (_HALF_ULP16 * abs(x1) + 1e-6)
        lo = max(zstar - w, x0 - cx0)
        hi = min(zstar + w, x1 + cx1)
        if hi > lo:
            zlo[r], zhi[r] = lo, hi
    return zlo, zhi


def _ref_cols(z3, eta_g):
    """Exact reference math for selected columns: z3 [R, ncols],
    eta_g [ncols, 4] -> [R, ncols] f32."""
    y0 = eta_g[:, 0]
    y1 = eta_g[:, 1]
    x0 = eta_g[:, 2]
    x1 = eta_g[:, 3]
    with np.errstate(divide="ignore", invalid="ignore"):
        lin = y0 + (y1 - y0) / (x1 - x0) * (z3 - x0)
    return np.where(z3 < x0, y0, np.where(z3 <= x1, lin, y1)).astype(np.float32)


def kernel(z, Mask, eta_fault):
    z = np.ascontiguousarray(np.asarray(z, dtype=np.float32))
    Mask = np.asarray(Mask)
    eta = np.asarray(eta_fault, dtype=np.float32)

    if "nc" not in _nc_cache:
        _nc_cache["nc"] = _build_nc()
    nc = _nc_cache["nc"]

    in_maps, (kcols, extra, const_bf, bad_bf) = make_in_maps(z, Mask, eta)
    mask_i = Mask.astype(np.int64)
    eta_bf = eta[mask_i]                             # [B, F, 4] f32

    res = run_bass_kernel_spmd(nc, in_maps, list(range(NCORES)))

    # Constant-output columns: fill y0 (broadcast over all rows).
    y0_bf = eta_bf[:, :, 0]                          # [B, F]
    out = np.empty((B, N, M, F), dtype=np.float32)
    for b in range(B):
        cb = const_bf[b]
        out[b, :, :, cb] = y0_bf[b, cb][:, None, None]

    NP0 = (NG - 1) * P                               # cols in out0
    for core in range(NCORES):
        b, nh = core // 2, core % 2
        k = kcols[b]
        view = out[b, nh * NH:(nh + 1) * NH].reshape(ROWS, F)
        arr0 = _unshard(res.results[core]["out0"])   # [ROWS, 768]
        view[:, k[:NP0]] = arr0[:, :min(k.size, NP0)]
        if k.size > NP0:
            arr1 = _unshard(res.results[core]["out1"])  # [ROWS, 96]
            view[:, k[NP0:]] = arr1[:, :k.size - NP0]
        if extra[b].size:                            # F2 overflow: host math
            zs = z[b, nh * NH:(nh + 1) * NH].reshape(ROWS, F)
            view[:, extra[b]] = _ref_cols(zs[:, extra[b]], eta_bf[b, extra[b]])

    # Exact host recomputation where the fp16 z rounding could matter:
    # elements whose z falls in the per-column risk band around the clip
    # knee (|out| small there). ~1.6% of elements for the standard table.
    zlo_r, zhi_r = _risk_bands(eta)
    zlo_bf = zlo_r[mask_i]                           # [B, F]
    zhi_bf = zhi_r[mask_i]
    risky = (z > zlo_bf[:, None, None, :]) & (z < zhi_bf[:, None, None, :])
    risky &= ~const_bf[:, None, None, :]
    if bad_bf.any():
        risky &= ~bad_bf[:, None, None, :]
    if risky.any():
        bi, _, _, fi = np.nonzero(risky)
        zv = z[risky]                                # f32, C-order like nonzero
        eg = eta_bf[bi, fi]                          # [K, 4] f32
        y0 = eg[:, 0]
        y1 = eg[:, 1]
        x0 = eg[:, 2]
        x1 = eg[:, 3]
        with np.errstate(divide="ignore", invalid="ignore"):
            lin = y0 + (y1 - y0) / (x1 - x0) * (zv - x0)
        out[risky] = np.where(
            zv < x0, y0, np.where(zv <= x1, lin, y1)).astype(np.float32)

    # Host patch for degenerate rows (never triggers with the standard table).
    if bad_bf.any():
        for b in range(B):
            (fbad,) = np.nonzero(bad_bf[b])
            if fbad.size == 0:
                continue
            zb = z[b][:, :, fbad]                    # [N, M, nbad]
            out[b][:, :, fbad] = _ref_cols(
                zb.reshape(-1, fbad.size), eta_bf[b, fbad]
            ).reshape(N, M, fbad.size)

    return out
